# revision 40
# baseline (speedup 1.0000x reference)
"""BitMLP (BitNet-style MLP) Trainium2 kernel, 8-way data-parallel over tokens.

reference semantics:
  h   = act_quant(rms_norm(x, gamma)) @ w1q.T   (w1q = per-tensor ternary quant)
  out = act_quant(gelu_exact(h)) @ w2q.T

Key facts exploited:
  * act_quant produces n/scale with n an integer in [-127, 127]  -> n is exact in bf16
  * weight quant produces t*inv_w with t ternary in {-1, 0, 1}   -> t is exact in bf16
  * therefore both matmuls are exact integer accumulations computed in bf16 at
    full TensorE rate; per-token/per-tensor scales are applied afterwards.

Sharding (8 cores on one chip):
  * tokens (4*2048 = 8192) split 1024/core; each core computes its tokens' output
  * weight quantization is cooperative: core c quantizes 1/8 of w1 and w2,
    the per-tensor mean(|w|) is combined with a scalar AllReduce, and the
    ternary bf16 weights are AllGathered.

Final schedule (2212us baseline -> 1503us), the key lessons paid for in traces:
  * every engine queue is IN-ORDER: a semaphore wait at the head blocks the
    whole queue, so emission order per engine is scheduled explicitly.
    Collective triggers, w2-quant loads/stores and out-stores live on gpsimd;
    stats + chunk + wb/wv loads split across sync/scalar/gpsimd DMA queues.
  * phase A: w1/w2/x stats stream concurrently on three queues; the 2-scalar
    AllReduce (per-tensor mean|w|) triggers ~84us in; w1 ternarization is
    2 scalar activation ops (Copy w*s+MAGIC, Copy -MAGIC) + 1 vector clip,
    chunked [4,2,2] hid-blocks/core and AllGathered so MM1 starts while
    chunks 1,2 are still gathering; the w2 AllGather is pinned after w1's
    via a data dep through t1_g[2].
  * MM1 token-half-outer (A then B): h(A) absmax rows + requant Q2(A) hide
    under MM1(B), Q2(B) under MM2(A); no PE stall at phase boundaries.
    MM2 d-pair-outer per half; weight streams alternate scalar/sync queues.
  * absmax rows round-trips (m2 -> DRAM -> row) go on the SYNC queue so they
    cannot be delayed by the w2-AG pin chain on gpsimd.
  * remaining gap to the ~900us PE roofline is mostly DVFS throttling
    (throttle_active ~240-1000us depending on schedule density) plus the
    serial CC stream (AR + 3 AGs + w2 AG ~ 370us, partly hidden).
"""

import os
import sys

for _p in ("/root/.axon_site/_ro/trn_rl_repo", "/opt/trn_rl_repo"):
    if os.path.isdir(_p) and _p not in sys.path:
        sys.path.append(_p)

from contextlib import ExitStack

import numpy as np

from concourse import bacc, bass, masks, mybir, tile
from concourse import bass_utils

F32 = mybir.dt.float32
BF16 = mybir.dt.bfloat16
AF = mybir.ActivationFunctionType
OP = mybir.AluOpType
AX = mybir.AxisListType

NCORES = 8
B, S, DIM, HID = 4, 2048, 2048, 8192
NTOK = B * S            # 8192
TOK = NTOK // NCORES    # 1024 tokens per core
KT = DIM // 128         # 16 k-tiles
HB = HID // 128         # 64 hid blocks
DB = DIM // 128         # 16 dim blocks
HBL = HID // NCORES // 128  # 8 hid blocks owned per core
CHUNKS = [4, 2, 2]      # w1 AG chunk sizes (hid blocks per core)
OFFS = [0, 4, 6]
NAG = len(CHUNKS)
MAGIC = 12582912.0      # 1.5 * 2**23: (v + MAGIC) - MAGIC == round-half-even(v)
EPS = 1e-6
W_NELEM = float(DIM * HID)

_cache = {}


def _build(n_cores=NCORES):
    nc = bacc.Bacc("TRN2", target_bir_lowering=False, debug=False, num_devices=n_cores)
    xT = nc.dram_tensor("xT", [DIM, TOK], F32, kind="ExternalInput")
    w1s = nc.dram_tensor("w1s", [DIM, HID // n_cores], F32, kind="ExternalInput")
    w2s = nc.dram_tensor("w2s", [HID // n_cores, DIM], F32, kind="ExternalInput")
    gpt = nc.dram_tensor("gpt", [128, KT], F32, kind="ExternalInput")
    outT = nc.dram_tensor("outT", [DIM, TOK], F32, kind="ExternalOutput")
    rg = [list(range(n_cores))]

    with tile.TileContext(nc) as tc, ExitStack() as ctx:
        misc = ctx.enter_context(tc.tile_pool(name="misc", bufs=1))
        xq = ctx.enter_context(tc.tile_pool(name="xq", bufs=1))
        hp0 = ctx.enter_context(tc.tile_pool(name="hp0", bufs=1))
        pw = ctx.enter_context(tc.tile_pool(name="pw", bufs=3))
        psc = ctx.enter_context(tc.tile_pool(name="psc", bufs=2))
        pmm2w = ctx.enter_context(tc.tile_pool(name="pmm2w", bufs=4))
        ps_mm = ctx.enter_context(tc.tile_pool(name="ps_mm", bufs=4, space="PSUM"))
        ps_tr = ctx.enter_context(tc.tile_pool(name="ps_tr", bufs=2, space="PSUM"))
        ps_ss = ctx.enter_context(tc.tile_pool(name="ps_ss", bufs=1, space="PSUM"))
        dram = ctx.enter_context(tc.tile_pool(name="dram", bufs=1, space="DRAM"))

        ident = misc.tile([128, 128], F32)
        masks.make_identity(nc, ident[:])
        zero_col = misc.tile([128, 1], F32)
        nc.vector.memset(zero_col[:], 0.0)
        ones_row = misc.tile([1, 128], F32)
        nc.vector.memset(ones_row[:], 1.0)
        ones_bf = misc.tile([128, 1], BF16)
        nc.vector.memset(ones_bf[:], 1.0)
        ones_f = misc.tile([128, 1], F32)
        nc.vector.memset(ones_f[:], 1.0)
        # persistent scale rows / broadcast tiles
        s1r = misc.tile([128, TOK], F32)        # (invw1 * inv_sx) per token
        s2r = misc.tile([128, TOK], F32)        # s2 per token, both halves
        i2r = misc.tile([128, TOK], F32)        # invw2 * inv_s2 per token
        invw = misc.tile([1, 2], F32)
        swb = misc.tile([128, 2], F32)
        gam = misc.tile([128, KT], F32)
        acc = misc.tile([128, TOK], F32)        # absmax accumulator (reused per phase)
        S1c = misc.tile([128, 2 * KT], F32)
        S2c = misc.tile([128, KT], F32)
        S12 = misc.tile([128, 2], F32)
        tot_sb = misc.tile([2, 1], F32)
        m0t = misc.tile([128, 8], F32)
        m2t = misc.tile([128, 4], F32)
        pin_t = misc.tile([2, 1], BF16)
        pin_d = misc.tile([2, 1], BF16)
        pin_z = misc.tile([2, 1], BF16)

        def bcast_row(dst, src_row, n, off=0):
            """dst[128, off:off+n] = broadcast of src_row[1, n] via PE outer product."""
            for o in range(0, n, 512):
                w = min(512, n - o)
                ps = ps_mm.tile([128, 512], F32, tag="mm")
                nc.tensor.matmul(ps[:, 0:w], ones_row[:], src_row[:, o:o + w],
                                 start=True, stop=True)
                nc.scalar.activation(dst[:, off + o:off + o + w], ps[:, 0:w], AF.Copy, bias=0.0)

        # DRAM scratch
        ar_in = dram.tile([2, 1], F32)
        ar_out = dram.tile([2, 1], F32, addr_space="Shared")
        t1_store = [dram.tile([CHUNKS[i], 128, KT * 128], BF16, name=f"t1s{i}")
                    for i in range(NAG)]
        t1_g = [dram.tile([n_cores, CHUNKS[i], 128, KT * 128], BF16, addr_space="Shared",
                          name=f"t1g{i}") for i in range(NAG)]
        t2_store = dram.tile([DB, 128, HBL * 128], BF16)
        t2_g = dram.tile([n_cores, DB, 128, HBL * 128], BF16, addr_space="Shared")
        r1_d = dram.tile([8, 128], F32)
        r2_d = [dram.tile([4, 128], F32, name=f"r2d{t}") for t in range(2)]

        nc.sync.dma_start(gam[:], gpt[:])

        nxT = xq.tile([128, KT * TOK], BF16)
        h0 = hp0.tile([128, HB * 512], BF16)

        with ExitStack() as sa:
            big = sa.enter_context(tc.tile_pool(name="big", bufs=2))     # w2/x/xre
            wio2 = sa.enter_context(tc.tile_pool(name="wio2", bufs=2))   # w1 stats + w2q loads
            scx2 = sa.enter_context(tc.tile_pool(name="scx2", bufs=1))
            xgap = sa.enter_context(tc.tile_pool(name="xgap", bufs=1))
            fsc = sa.enter_context(tc.tile_pool(name="fsc", bufs=2))     # f32 scratch ring
            wio = sa.enter_context(tc.tile_pool(name="wio", bufs=3))     # w1 chunk col loads
            qio = sa.enter_context(tc.tile_pool(name="qio", bufs=2))     # w1 ternary bf16
            qio2 = sa.enter_context(tc.tile_pool(name="qio2", bufs=1))   # w2 ternary bf16
            rring = sa.enter_context(tc.tile_pool(name="rring", bufs=3))
            sax = sa.enter_context(tc.tile_pool(name="sax", bufs=1))

            rsx = sax.tile([128, TOK], F32)
            inv_sx = sax.tile([1, TOK], F32)
            rscr = sax.tile([1, TOK], F32)
            xga = xgap.tile([128, TOK], F32)

            nc.vector.memset(acc[:], 0.0)

            # ---- |w1| stats: full-row loads split across sync + gpsimd ------
            for kt in range(KT):
                wt = wio2.tile([128, TOK], F32, tag="w2")
                eng = nc.sync if kt % 2 == 0 else nc.gpsimd
                eng.dma_start(wt[:], w1s[kt * 128:(kt + 1) * 128, :])
                nc.vector.tensor_reduce(S1c[:, kt:kt + 1], wt[:], axis=AX.X, op=OP.add,
                                        apply_absolute_value=True)
            for ht in range(HBL):
                for hf in range(2):
                    w2t = big.tile([128, TOK], F32, tag="big")
                    nc.scalar.dma_start(w2t[:], w2s[ht * 128:(ht + 1) * 128,
                                                    hf * 1024:(hf + 1) * 1024])
                    nc.vector.tensor_reduce(S2c[:, 2 * ht + hf:2 * ht + hf + 1], w2t[:],
                                            axis=AX.X, op=OP.add,
                                            apply_absolute_value=True)
            nc.vector.tensor_reduce(S12[:, 0:1], S1c[:, 0:KT], axis=AX.X, op=OP.add)
            nc.vector.tensor_reduce(S12[:, 1:2], S2c[:], axis=AX.X, op=OP.add)
            tot_ps = ps_tr.tile([2, 1], F32, tag="tr")
            nc.tensor.matmul(tot_ps[:], S12[:], ones_f[:], start=True, stop=True)
            nc.vector.tensor_copy(tot_sb[:], tot_ps[:])
            nc.gpsimd.dma_start(ar_in[:], tot_sb[:])
            nc.gpsimd.collective_compute(
                "AllReduce", OP.add, replica_groups=rg, ins=[ar_in[:]], outs=[ar_out[:]])

            # ---- AllReduce result -> weight scales (gpsimd queue: the sync
            # queue must not stall on AR completion ahead of the chunk loads) -
            tot2 = rring.tile([1, TOK], F32, tag="row")
            nc.gpsimd.dma_start(tot2[:, 0:2], ar_out[:].rearrange("a b -> b a"))
            nc.vector.tensor_scalar(invw[:], tot2[:, 0:2], 1.0 / W_NELEM, 1e-5,
                                    op0=OP.mult, op1=OP.max)
            sw = rring.tile([1, TOK], F32, tag="row")
            nc.vector.reciprocal(sw[:, 0:2], invw[:])
            ps_b = ps_tr.tile([128, 2], F32, tag="tr")
            nc.tensor.matmul(ps_b[:], ones_row[:], sw[:, 0:2], start=True, stop=True)
            nc.scalar.activation(swb[:], ps_b[:], AF.Copy, bias=0.0)

            def w1_chunk_quant(ci, kt):
                CH = CHUNKS[ci]
                CW = CH * 128
                wq = wio.tile([128, 4 * 128], F32, tag="wq")
                nc.sync.dma_start(wq[:, 0:CW], w1s[kt * 128:(kt + 1) * 128,
                                                   OFFS[ci] * 128:OFFS[ci] * 128 + CW])
                wf = fsc.tile([128, TOK], F32, tag="fsc")
                nc.scalar.activation(wf[:, 0:CW], wq[:, 0:CW], AF.Copy,
                                     bias=MAGIC, scale=swb[:, 0:1])
                q = qio.tile([128, 4 * 128], BF16, tag="q")
                nc.scalar.activation(q[:, 0:CW], wf[:, 0:CW], AF.Copy, bias=-MAGIC)
                nc.vector.tensor_scalar(q[:, 0:CW], q[:, 0:CW], 1.0, -1.0,
                                        op0=OP.min, op1=OP.max)
                nc.gpsimd.dma_start(
                    t1_store[ci][:, :, kt * 128:(kt + 1) * 128].rearrange(
                        "b k j -> k b j"),
                    q[:, 0:CW].rearrange("k (b j) -> k b j", b=CH))

            # ---- chunk 0 quant interleaved with x loads + sum-of-squares ----
            ss_ps0 = ps_ss.tile([1, 512], F32, tag="ss0")
            ss_ps1 = ps_ss.tile([1, 512], F32, tag="ss1")
            xts = []
            for kt in range(KT):
                w1_chunk_quant(0, kt)
                xt = big.tile([128, TOK], F32, tag="big")
                nc.gpsimd.dma_start(xt[:], xT[kt * 128:(kt + 1) * 128, :])
                xts.append(xt)
                x2 = scx2.tile([128, TOK], BF16, tag="x2")
                nc.vector.tensor_tensor(x2[:], xt[:], xt[:], op=OP.mult)
                nc.tensor.matmul(ss_ps0[:], ones_bf[:], x2[:, 0:512],
                                 start=(kt == 0), stop=(kt == KT - 1))
                nc.tensor.matmul(ss_ps1[:], ones_bf[:], x2[:, 512:1024],
                                 start=(kt == 0), stop=(kt == KT - 1))
            nc.gpsimd.collective_compute(
                "AllGather", OP.bypass, replica_groups=rg,
                ins=[t1_store[0][:]], outs=[t1_g[0][:]])
            # ---- chunks 1, 2 quant interleaved with |x*gam| absmax ----------
            for ci in (1, 2):
                for kt in range(KT):
                    w1_chunk_quant(ci, kt)
                    if kt % 2 == ci - 1:
                        k2a = (ci - 1) * 8 + kt // 2
                        nc.scalar.activation(xga[:], xts[k2a][:], AF.Abs,
                                             bias=zero_col[:],
                                             scale=gam[:, k2a:k2a + 1])
                        nc.vector.tensor_tensor(acc[:], acc[:], xga[:], op=OP.max)
                nc.gpsimd.collective_compute(
                    "AllGather", OP.bypass, replica_groups=rg,
                    ins=[t1_store[ci][:]], outs=[t1_g[ci][:]])

            # ---- token rows: rstd + absmax -> sx, rsx -----------------------
            v_row = rring.tile([1, TOK], F32, tag="row")
            nc.vector.tensor_scalar(v_row[:, 0:512], ss_ps0[:], 1.0 / DIM, EPS,
                                    op0=OP.mult, op1=OP.add)
            nc.vector.tensor_scalar(v_row[:, 512:1024], ss_ps1[:], 1.0 / DIM, EPS,
                                    op0=OP.mult, op1=OP.add)
            sq_row = rring.tile([1, TOK], F32, tag="row")
            nc.scalar.activation(sq_row[:], v_row[:], AF.Sqrt, bias=zero_col[0:1, :])
            rstd_row = rring.tile([1, TOK], F32, tag="row")
            nc.vector.reciprocal_approx_accurate(rstd_row[:], sq_row[:], rscr[:])

            for c in range(8):
                pt = ps_tr.tile([128, 128], F32, tag="tr")
                nc.tensor.transpose(pt[:], acc[:, c * 128:(c + 1) * 128], ident[:])
                nc.vector.tensor_reduce(m0t[:, c:c + 1], pt[:], axis=AX.X, op=OP.max)
            nc.gpsimd.dma_start(r1_d[:].rearrange("c p -> p c"), m0t[:])
            m0row = rring.tile([1, TOK], F32, tag="row")
            nc.gpsimd.dma_start(m0row[:], r1_d[:].rearrange("c p -> (c p)")[None, :])
            nc.vector.tensor_tensor(m0row[:], m0row[:], rstd_row[:], op=OP.mult)
            nc.vector.tensor_scalar(m0row[:], m0row[:], 1e-5, None, op0=OP.max)
            sx_row = rring.tile([1, TOK], F32, tag="row")
            nc.vector.reciprocal_approx_accurate(sx_row[:], m0row[:], rscr[:])
            nc.vector.tensor_scalar(sx_row[:], sx_row[:], 127.0, None, op0=OP.mult)
            nc.vector.reciprocal_approx_accurate(inv_sx[:], sx_row[:], rscr[:])
            nc.vector.tensor_tensor(rstd_row[:], rstd_row[:], sx_row[:], op=OP.mult)
            bcast_row(rsx, rstd_row, TOK)

            # ---- quantize x: n_xT = round((x*gam)*rsx) ----------------------
            for kt in range(KT):
                xr = big.tile([128, TOK], F32, tag="big")
                nc.sync.dma_start(xr[:], xT[kt * 128:(kt + 1) * 128, :])
                t = fsc.tile([128, TOK], F32, tag="fsc")
                nc.scalar.activation(t[:], xr[:], AF.Copy, bias=0.0,
                                     scale=gam[:, kt:kt + 1])
                nc.vector.tensor_tensor(t[:], t[:], rsx[:], op=OP.mult)
                nc.vector.tensor_scalar(nxT[:, kt * TOK:(kt + 1) * TOK], t[:], MAGIC, MAGIC,
                                        op0=OP.add, op1=OP.subtract)

            # ---- s1 row: invw1 * inv_sx -------------------------------------
            s1_row = rring.tile([1, TOK], F32, tag="row")
            nc.vector.tensor_scalar(s1_row[:], inv_sx[:], invw[:, 0:1], None, op0=OP.mult)
            bcast_row(s1r, s1_row, TOK)
            # reset absmax accumulator for the h phase (accA | accB halves)
            nc.vector.memset(acc[:], 0.0)

            # ---- w2 quant: loads on gpsimd, scalar round, vec clip ----------
            for ht in range(HBL):
                for hf in range(2):
                    w2l = wio2.tile([128, TOK], F32, tag="w2")
                    nc.gpsimd.dma_start(w2l[:], w2s[ht * 128:(ht + 1) * 128,
                                                    hf * 1024:(hf + 1) * 1024])
                    wf2 = fsc.tile([128, TOK], F32, tag="fsc")
                    nc.scalar.activation(wf2[:], w2l[:], AF.Copy,
                                         bias=MAGIC, scale=swb[:, 1:2])
                    q2 = qio2.tile([128, TOK], BF16, tag="q2")
                    nc.scalar.activation(q2[:], wf2[:], AF.Copy, bias=-MAGIC)
                    nc.vector.tensor_scalar(q2[:], q2[:], 1.0, -1.0,
                                            op0=OP.min, op1=OP.max)
                    d0 = hf * 8
                    nc.gpsimd.dma_start(
                        t2_store[d0:d0 + 8, :, ht * 128:(ht + 1) * 128].rearrange(
                            "d k j -> k d j"),
                        q2[:].rearrange("k (d j) -> k d j", d=8))
            # pin: last write into t2_store is data-dependent on t1_g[2] (AG2
            # output), so the w2 AllGather cannot be scheduled before w1's AGs.
            nc.gpsimd.dma_start(pin_t[:], t2_store[0, 0:2, 0:1])
            nc.gpsimd.dma_start(pin_d[:], t1_g[NAG - 1][0, 0, 0:2, 0:1])
            nc.gpsimd.tensor_scalar(pin_z[:], pin_d[:], 0.0, None, op0=OP.mult)
            nc.gpsimd.tensor_tensor(pin_z[:], pin_z[:], pin_t[:], op=OP.add)
            nc.gpsimd.dma_start(t2_store[0, 0:2, 0:1], pin_z[:])
            nc.gpsimd.collective_compute(
                "AllGather", OP.bypass, replica_groups=rg, ins=[t2_store[:]], outs=[t2_g[:]])

            # ============ MM1, token half A (all 64 hid blocks) ==============
            # blocks processed in PAIRS with two PSUM banks alternating per
            # matmul instruction: back-to-back accumulation into one bank
            # stalls the PE pipe (~400ns/mm vs ~190ns with alternation).
            blocks = [(ci, r, bi) for ci in range(NAG) for r in range(n_cores)
                      for bi in range(CHUNKS[ci])]
            pairs = [(blocks[2 * i], blocks[2 * i + 1]) for i in range(len(blocks) // 2)]

            def mm1_pair(blkA, blkB, th, htile):
                to = th * 512
                wbt = []
                for (ci, r, bi) in (blkA, blkB):
                    wb = pw.tile([128, KT * 128], BF16, tag="wb")
                    nc.sync.dma_start(wb[:], t1_g[ci][r, bi])
                    wbt.append(wb)
                psA = ps_mm.tile([128, 512], F32, tag="mm")
                psB = ps_mm.tile([128, 512], F32, tag="mm")
                for kt in range(KT):
                    st, sp = (kt == 0), (kt == KT - 1)
                    mv = nxT[:, kt * TOK + to:kt * TOK + to + 512]
                    nc.tensor.matmul(psA[:], wbt[0][:, kt * 128:(kt + 1) * 128], mv,
                                     start=st, stop=sp)
                    nc.tensor.matmul(psB[:], wbt[1][:, kt * 128:(kt + 1) * 128], mv,
                                     start=st, stop=sp)
                for (ci, r, bi), ps in ((blkA, psA), (blkB, psB)):
                    ghb = r * HBL + OFFS[ci] + bi
                    hs = psc.tile([128, 512], F32, tag="hs")
                    nc.vector.tensor_tensor(hs[:], ps[:], s1r[:, to:to + 512], op=OP.mult)
                    hsl = htile[:, ghb * 512:(ghb + 1) * 512]
                    nc.scalar.activation(hsl, hs[:], AF.Gelu, bias=zero_col[:])
                    ga = psc.tile([128, 512], BF16, tag="ga")
                    nc.scalar.activation(ga[:], hsl, AF.Abs, bias=zero_col[:])
                    nc.vector.tensor_tensor(acc[:, to:to + 512], acc[:, to:to + 512],
                                            ga[:], op=OP.max)

            for (blkA, blkB) in pairs:
                mm1_pair(blkA, blkB, 0, h0)

        # ---- phase A scratch pool closed; h1 + late rows live in its space --
        with ExitStack() as sb:
            hp1 = sb.enter_context(tc.tile_pool(name="hp1", bufs=1))
            rowp = sb.enter_context(tc.tile_pool(name="rowp", bufs=1))
            h1 = hp1.tile([128, HB * 512], BF16)
            # vector-only scratch rows, both on partition 0 (engine AP rule);
            # the s2/i2 rows themselves are computed into partition 0 of their
            # broadcast tiles.
            lrows = rowp.tile([1, 1024], F32)

            def half_rows(th):
                """acc[:, th*512:+512] absmax -> s2row/i2row for that half."""
                to = th * 512
                for c in range(4):
                    pt = ps_tr.tile([128, 128], F32, tag="tr")
                    nc.tensor.transpose(pt[:], acc[:, to + c * 128:to + (c + 1) * 128],
                                        ident[:])
                    nc.vector.tensor_reduce(m2t[:, c:c + 1], pt[:], axis=AX.X, op=OP.max)
                nc.sync.dma_start(r2_d[th][:].rearrange("c p -> p c"), m2t[:])
                m2row = lrows[0:1, 0:512]
                nc.sync.dma_start(m2row, r2_d[th][:].rearrange("c p -> (c p)")[None, :])
                nc.vector.tensor_scalar(m2row, m2row, 1e-5, None, op0=OP.max)
                rs2 = lrows[0:1, 512:1024]
                s2row = s2r[0:1, to:to + 512]
                nc.vector.reciprocal_approx_accurate(s2row, m2row, rs2)
                nc.vector.tensor_scalar(s2row, s2row, 127.0, None, op0=OP.mult)
                i2row = i2r[0:1, to:to + 512]
                nc.vector.reciprocal_approx_accurate(i2row, s2row, rs2)
                nc.vector.tensor_scalar(i2row, i2row, invw[:, 1:2], None, op0=OP.mult)
                return s2row, i2row

            def half_bcast(th, s2row, i2row):
                to = th * 512
                bcast_row(s2r, s2row, 512, off=to)
                bcast_row(i2r, i2row, 512, off=to)

            def q2_block(th, htile, kg):
                """requant h in place: vector scale-mult, scalar MAGIC round."""
                to = th * 512
                hsl = htile[:, kg * 512:(kg + 1) * 512]
                t2s = psc.tile([128, 512], F32, tag="hs")
                nc.vector.tensor_tensor(t2s[:], hsl, s2r[:, to:to + 512], op=OP.mult)
                rq = psc.tile([128, 512], F32, tag="hs")
                nc.scalar.activation(rq[:], t2s[:], AF.Copy, bias=MAGIC)
                nc.scalar.activation(hsl, rq[:], AF.Copy, bias=-MAGIC)

            # ============ MM1 half B, with rows(A) + Q2(A) hidden under it ===
            q2a = 0
            rowsA = [None]
            for idx, (blkA, blkB) in enumerate(pairs):
                mm1_pair(blkA, blkB, 1, h1)
                if idx == 2:
                    rowsA[0] = half_rows(0)
                elif idx == 4:
                    half_bcast(0, *rowsA[0])
                elif idx >= 5 and q2a < HB:
                    for _ in range(3):
                        if q2a < HB:
                            q2_block(0, h0, q2a)
                            q2a += 1
            while q2a < HB:
                q2_block(0, h0, q2a)
                q2a += 1

            # ============ MM2 per half, d-pairs with alternating PSUM banks ==
            def mm2_dpair(th, htile, d0):
                to = th * 512
                psA = ps_mm.tile([128, 512], F32, tag="mm")
                psB = ps_mm.tile([128, 512], F32, tag="mm")
                for r in range(n_cores):
                    wv0 = pmm2w.tile([128, HBL * 128], BF16, tag="wv")
                    nc.scalar.dma_start(wv0[:], t2_g[r, d0])
                    wv1 = pmm2w.tile([128, HBL * 128], BF16, tag="wv")
                    nc.sync.dma_start(wv1[:], t2_g[r, d0 + 1])
                    for k2 in range(HBL):
                        kg = r * HBL + k2
                        st, sp = (kg == 0), (kg == HB - 1)
                        mv = htile[:, kg * 512:(kg + 1) * 512]
                        nc.tensor.matmul(psA[:], wv0[:, k2 * 128:(k2 + 1) * 128], mv,
                                         start=st, stop=sp)
                        nc.tensor.matmul(psB[:], wv1[:, k2 * 128:(k2 + 1) * 128], mv,
                                         start=st, stop=sp)
                for d, ps in ((d0, psA), (d0 + 1, psB)):
                    ot = psc.tile([128, 512], F32, tag="hs")
                    nc.vector.tensor_tensor(ot[:], ps[:], i2r[:, to:to + 512], op=OP.mult)
                    nc.gpsimd.dma_start(outT[d * 128:(d + 1) * 128, to:to + 512], ot[:])

            q2b = 0
            rowsB = [None]
            for dp in range(DB // 2):
                mm2_dpair(0, h0, 2 * dp)
                if dp == 0:
                    rowsB[0] = half_rows(1)
                elif dp == 1:
                    half_bcast(1, *rowsB[0])
                elif q2b < HB:
                    for _ in range(11):
                        if q2b < HB:
                            q2_block(1, h1, q2b)
                            q2b += 1
            while q2b < HB:
                q2_block(1, h1, q2b)
                q2b += 1

            for dp in range(DB // 2):
                mm2_dpair(1, h1, 2 * dp)

    nc.compile()
    return nc


def _get_nc():
    if "nc" not in _cache:
        _cache["nc"] = _build()
    return _cache["nc"]


def _prep_inputs(x, w1, w2, gamma):
    x2d = np.ascontiguousarray(np.asarray(x, dtype=np.float32).reshape(NTOK, DIM))
    w1 = np.asarray(w1, dtype=np.float32)
    w2 = np.asarray(w2, dtype=np.float32)
    gamma = np.asarray(gamma, dtype=np.float32)
    w1T = np.ascontiguousarray(w1.T)          # [DIM, HID]
    w2T = np.ascontiguousarray(w2.T)          # [HID, DIM]
    gpt = np.ascontiguousarray(gamma.reshape(KT, 128).T)
    hs = HID // NCORES
    in_maps = []
    for c in range(NCORES):
        in_maps.append({
            "xT": np.ascontiguousarray(x2d[c * TOK:(c + 1) * TOK, :].T),
            "w1s": np.ascontiguousarray(w1T[:, c * hs:(c + 1) * hs]),
            "w2s": np.ascontiguousarray(w2T[c * hs:(c + 1) * hs, :]),
            "gpt": gpt,
        })
    return in_maps


def _run(in_maps, trace=False, **kw):
    nc = _get_nc()
    return bass_utils.run_bass_kernel_spmd(
        nc, in_maps, core_ids=list(range(NCORES)), trace=trace, **kw)


def kernel(x, w1, w2, gamma):
    in_maps = _prep_inputs(x, w1, w2, gamma)
    res = _run(in_maps, trace=False)
    out = np.empty((NTOK, DIM), dtype=np.float32)
    for c in range(NCORES):
        out[c * TOK:(c + 1) * TOK, :] = res.results[c]["outT"].T
    return out.reshape(B, S, DIM)


# revision 46
# speedup vs baseline: 1.0621x; 1.0621x over previous
"""BitMLP (BitNet-style MLP) Trainium2 kernel, 8-way data-parallel over tokens.

reference semantics:
  h   = act_quant(rms_norm(x, gamma)) @ w1q.T   (w1q = per-tensor ternary quant)
  out = act_quant(gelu_exact(h)) @ w2q.T

Key facts exploited:
  * act_quant produces n/scale with n an integer in [-127, 127]  -> n is exact in bf16
  * weight quant produces t*inv_w with t ternary in {-1, 0, 1}   -> t is exact in bf16
  * therefore both matmuls are exact integer accumulations computed in bf16 at
    full TensorE rate; per-token/per-tensor scales are applied afterwards.

Sharding (8 cores on one chip):
  * tokens (4*2048 = 8192) split 1024/core; each core computes its tokens' output
  * weight quantization is cooperative: core c quantizes 1/8 of w1 and w2,
    the per-tensor mean(|w|) is combined with a scalar AllReduce, and the
    ternary bf16 weights are AllGathered.

Final schedule (2212us baseline -> 1503us), the key lessons paid for in traces:
  * every engine queue is IN-ORDER: a semaphore wait at the head blocks the
    whole queue, so emission order per engine is scheduled explicitly.
    Collective triggers, w2-quant loads/stores and out-stores live on gpsimd;
    stats + chunk + wb/wv loads split across sync/scalar/gpsimd DMA queues.
  * phase A: w1/w2/x stats stream concurrently on three queues; the 2-scalar
    AllReduce (per-tensor mean|w|) triggers ~84us in; w1 ternarization is
    2 scalar activation ops (Copy w*s+MAGIC, Copy -MAGIC) + 1 vector clip,
    chunked [4,2,2] hid-blocks/core and AllGathered so MM1 starts while
    chunks 1,2 are still gathering; the w2 AllGather is pinned after w1's
    via a data dep through t1_g[2].
  * MM1 token-half-outer (A then B): h(A) absmax rows + requant Q2(A) hide
    under MM1(B), Q2(B) under MM2(A); no PE stall at phase boundaries.
    MM2 d-pair-outer per half; weight streams alternate scalar/sync queues.
  * absmax rows round-trips (m2 -> DRAM -> row) go on the SYNC queue so they
    cannot be delayed by the w2-AG pin chain on gpsimd.
  * remaining gap to the ~900us PE roofline is mostly DVFS throttling
    (throttle_active ~240-1000us depending on schedule density) plus the
    serial CC stream (AR + 3 AGs + w2 AG ~ 370us, partly hidden).
"""

import os
import sys

for _p in ("/root/.axon_site/_ro/trn_rl_repo", "/opt/trn_rl_repo"):
    if os.path.isdir(_p) and _p not in sys.path:
        sys.path.append(_p)

from contextlib import ExitStack

import numpy as np

from concourse import bacc, bass, masks, mybir, tile
from concourse import bass_utils

F32 = mybir.dt.float32
BF16 = mybir.dt.bfloat16
FP8 = mybir.dt.float8e4    # e4m3: ternary {-1,0,1} exact
AF = mybir.ActivationFunctionType
OP = mybir.AluOpType
AX = mybir.AxisListType

NCORES = 8
B, S, DIM, HID = 4, 2048, 2048, 8192
NTOK = B * S            # 8192
TOK = NTOK // NCORES    # 1024 tokens per core
KT = DIM // 128         # 16 k-tiles
HB = HID // 128         # 64 hid blocks
DB = DIM // 128         # 16 dim blocks
HBL = HID // NCORES // 128  # 8 hid blocks owned per core
CHUNKS = [4, 2, 2]      # w1 AG chunk sizes (hid blocks per core)
OFFS = [0, 4, 6]
NAG = len(CHUNKS)
MAGIC = 12582912.0      # 1.5 * 2**23: (v + MAGIC) - MAGIC == round-half-even(v)
EPS = 1e-6
W_NELEM = float(DIM * HID)

_cache = {}


def _build(n_cores=NCORES):
    nc = bacc.Bacc("TRN2", target_bir_lowering=False, debug=False, num_devices=n_cores)
    xT = nc.dram_tensor("xT", [DIM, TOK], F32, kind="ExternalInput")
    w1s = nc.dram_tensor("w1s", [DIM, HID // n_cores], F32, kind="ExternalInput")
    w2s = nc.dram_tensor("w2s", [HID // n_cores, DIM], F32, kind="ExternalInput")
    gpt = nc.dram_tensor("gpt", [128, KT], F32, kind="ExternalInput")
    outT = nc.dram_tensor("outT", [DIM, TOK], F32, kind="ExternalOutput")
    rg = [list(range(n_cores))]

    with tile.TileContext(nc) as tc, ExitStack() as ctx:
        misc = ctx.enter_context(tc.tile_pool(name="misc", bufs=1))
        xq = ctx.enter_context(tc.tile_pool(name="xq", bufs=1))
        hp0 = ctx.enter_context(tc.tile_pool(name="hp0", bufs=1))
        pw = ctx.enter_context(tc.tile_pool(name="pw", bufs=3))
        psc = ctx.enter_context(tc.tile_pool(name="psc", bufs=2))
        pmm2w = ctx.enter_context(tc.tile_pool(name="pmm2w", bufs=4))
        ps_mm = ctx.enter_context(tc.tile_pool(name="ps_mm", bufs=4, space="PSUM"))
        ps_tr = ctx.enter_context(tc.tile_pool(name="ps_tr", bufs=2, space="PSUM"))
        ps_ss = ctx.enter_context(tc.tile_pool(name="ps_ss", bufs=1, space="PSUM"))
        dram = ctx.enter_context(tc.tile_pool(name="dram", bufs=1, space="DRAM"))

        ident = misc.tile([128, 128], F32)
        masks.make_identity(nc, ident[:])
        zero_col = misc.tile([128, 1], F32)
        nc.vector.memset(zero_col[:], 0.0)
        ones_row = misc.tile([1, 128], F32)
        nc.vector.memset(ones_row[:], 1.0)
        ones_bf = misc.tile([128, 1], BF16)
        nc.vector.memset(ones_bf[:], 1.0)
        ones_f = misc.tile([128, 1], F32)
        nc.vector.memset(ones_f[:], 1.0)
        # persistent scale rows / broadcast tiles
        s1r = misc.tile([128, TOK], F32)        # (invw1 * inv_sx) per token
        s2r = misc.tile([128, TOK], F32)        # s2 per token, both halves
        i2r = misc.tile([128, TOK], F32)        # invw2 * inv_s2 per token
        invw = misc.tile([1, 2], F32)
        swb = misc.tile([128, 2], F32)
        gam = misc.tile([128, KT], F32)
        acc = misc.tile([128, TOK], F32)        # absmax accumulator (reused per phase)
        S1c = misc.tile([128, 2 * KT], F32)
        S2c = misc.tile([128, KT], F32)
        S12 = misc.tile([128, 2], F32)
        tot_sb = misc.tile([2, 1], F32)
        m0t = misc.tile([128, 8], F32)
        m2t = misc.tile([128, 4], F32)
        pin_t = misc.tile([2, 1], BF16)
        pin_d = misc.tile([2, 1], FP8)
        pin_zf = misc.tile([2, 1], F32)
        pin_z = misc.tile([2, 1], BF16)

        def bcast_row(dst, src_row, n, off=0):
            """dst[128, off:off+n] = broadcast of src_row[1, n] via PE outer product."""
            for o in range(0, n, 512):
                w = min(512, n - o)
                ps = ps_mm.tile([128, 512], F32, tag="mm")
                nc.tensor.matmul(ps[:, 0:w], ones_row[:], src_row[:, o:o + w],
                                 start=True, stop=True)
                nc.scalar.activation(dst[:, off + o:off + o + w], ps[:, 0:w], AF.Copy, bias=0.0)

        # DRAM scratch
        ar_in = dram.tile([2, 1], F32)
        ar_out = dram.tile([2, 1], F32, addr_space="Shared")
        t1_store = [dram.tile([CHUNKS[i], 128, KT * 128], FP8, name=f"t1s{i}")
                    for i in range(NAG)]
        t1_g = [dram.tile([n_cores, CHUNKS[i], 128, KT * 128], FP8, addr_space="Shared",
                          name=f"t1g{i}") for i in range(NAG)]
        t2_store = dram.tile([DB, 128, HBL * 128], BF16)
        t2_g = dram.tile([n_cores, DB, 128, HBL * 128], BF16, addr_space="Shared")
        r1_d = dram.tile([8, 128], F32)
        r2_d = [dram.tile([4, 128], F32, name=f"r2d{t}") for t in range(2)]

        nc.sync.dma_start(gam[:], gpt[:])

        nxT = xq.tile([128, KT * TOK], BF16)
        h0 = hp0.tile([128, HB * 512], BF16)

        with ExitStack() as sa:
            big = sa.enter_context(tc.tile_pool(name="big", bufs=2))     # w2/x/xre
            wio2 = sa.enter_context(tc.tile_pool(name="wio2", bufs=2))   # w1 stats + w2q loads
            scx2 = sa.enter_context(tc.tile_pool(name="scx2", bufs=1))
            xgap = sa.enter_context(tc.tile_pool(name="xgap", bufs=1))
            fsc = sa.enter_context(tc.tile_pool(name="fsc", bufs=2))     # f32 scratch ring
            wio = sa.enter_context(tc.tile_pool(name="wio", bufs=3))     # w1 chunk col loads
            qio = sa.enter_context(tc.tile_pool(name="qio", bufs=2))     # w1 ternary bf16
            qio2 = sa.enter_context(tc.tile_pool(name="qio2", bufs=1))   # w2 ternary bf16
            rring = sa.enter_context(tc.tile_pool(name="rring", bufs=3))
            sax = sa.enter_context(tc.tile_pool(name="sax", bufs=1))

            rsx = sax.tile([128, TOK], F32)
            inv_sx = sax.tile([1, TOK], F32)
            rscr = sax.tile([1, TOK], F32)
            xga = xgap.tile([128, TOK], F32)

            nc.vector.memset(acc[:], 0.0)

            # ---- |w1| stats: full-row loads split across sync + gpsimd ------
            for kt in range(KT):
                wt = wio2.tile([128, TOK], F32, tag="w2")
                eng = nc.sync if kt % 2 == 0 else nc.gpsimd
                eng.dma_start(wt[:], w1s[kt * 128:(kt + 1) * 128, :])
                nc.vector.tensor_reduce(S1c[:, kt:kt + 1], wt[:], axis=AX.X, op=OP.add,
                                        apply_absolute_value=True)
            for ht in range(HBL):
                for hf in range(2):
                    w2t = big.tile([128, TOK], F32, tag="big")
                    nc.scalar.dma_start(w2t[:], w2s[ht * 128:(ht + 1) * 128,
                                                    hf * 1024:(hf + 1) * 1024])
                    nc.vector.tensor_reduce(S2c[:, 2 * ht + hf:2 * ht + hf + 1], w2t[:],
                                            axis=AX.X, op=OP.add,
                                            apply_absolute_value=True)
            nc.vector.tensor_reduce(S12[:, 0:1], S1c[:, 0:KT], axis=AX.X, op=OP.add)
            nc.vector.tensor_reduce(S12[:, 1:2], S2c[:], axis=AX.X, op=OP.add)
            tot_ps = ps_tr.tile([2, 1], F32, tag="tr")
            nc.tensor.matmul(tot_ps[:], S12[:], ones_f[:], start=True, stop=True)
            nc.vector.tensor_copy(tot_sb[:], tot_ps[:])
            nc.gpsimd.dma_start(ar_in[:], tot_sb[:])
            nc.gpsimd.collective_compute(
                "AllReduce", OP.add, replica_groups=rg, ins=[ar_in[:]], outs=[ar_out[:]])

            # ---- AllReduce result -> weight scales (gpsimd queue: the sync
            # queue must not stall on AR completion ahead of the chunk loads) -
            tot2 = rring.tile([1, TOK], F32, tag="row")
            nc.gpsimd.dma_start(tot2[:, 0:2], ar_out[:].rearrange("a b -> b a"))
            nc.vector.tensor_scalar(invw[:], tot2[:, 0:2], 1.0 / W_NELEM, 1e-5,
                                    op0=OP.mult, op1=OP.max)
            sw = rring.tile([1, TOK], F32, tag="row")
            nc.vector.reciprocal(sw[:, 0:2], invw[:])
            ps_b = ps_tr.tile([128, 2], F32, tag="tr")
            nc.tensor.matmul(ps_b[:], ones_row[:], sw[:, 0:2], start=True, stop=True)
            nc.scalar.activation(swb[:], ps_b[:], AF.Copy, bias=0.0)

            def w1_chunk_quant(ci, kt):
                CH = CHUNKS[ci]
                CW = CH * 128
                wq = wio.tile([128, 4 * 128], F32, tag="wq")
                nc.sync.dma_start(wq[:, 0:CW], w1s[kt * 128:(kt + 1) * 128,
                                                   OFFS[ci] * 128:OFFS[ci] * 128 + CW])
                wf = fsc.tile([128, TOK], F32, tag="fsc")
                nc.scalar.activation(wf[:, 0:CW], wq[:, 0:CW], AF.Copy,
                                     bias=MAGIC, scale=swb[:, 0:1])
                # clip in the MAGIC domain (f32), then write the ternary as fp8
                nc.vector.tensor_scalar(wf[:, 0:CW], wf[:, 0:CW],
                                        MAGIC + 1.0, MAGIC - 1.0,
                                        op0=OP.min, op1=OP.max)
                q = qio.tile([128, 4 * 128], FP8, tag="q")
                nc.scalar.activation(q[:, 0:CW], wf[:, 0:CW], AF.Copy, bias=-MAGIC)
                nc.gpsimd.dma_start(
                    t1_store[ci][:, :, kt * 128:(kt + 1) * 128].rearrange(
                        "b k j -> k b j"),
                    q[:, 0:CW].rearrange("k (b j) -> k b j", b=CH))

            # ---- chunk 0 quant interleaved with x loads + sum-of-squares ----
            ss_ps0 = ps_ss.tile([1, 512], F32, tag="ss0")
            ss_ps1 = ps_ss.tile([1, 512], F32, tag="ss1")
            xts = []
            for kt in range(KT):
                w1_chunk_quant(0, kt)
                xt = big.tile([128, TOK], F32, tag="big")
                nc.gpsimd.dma_start(xt[:], xT[kt * 128:(kt + 1) * 128, :])
                xts.append(xt)
                x2 = scx2.tile([128, TOK], BF16, tag="x2")
                nc.vector.tensor_tensor(x2[:], xt[:], xt[:], op=OP.mult)
                nc.tensor.matmul(ss_ps0[:], ones_bf[:], x2[:, 0:512],
                                 start=(kt == 0), stop=(kt == KT - 1))
                nc.tensor.matmul(ss_ps1[:], ones_bf[:], x2[:, 512:1024],
                                 start=(kt == 0), stop=(kt == KT - 1))
            nc.gpsimd.collective_compute(
                "AllGather", OP.bypass, replica_groups=rg,
                ins=[t1_store[0][:]], outs=[t1_g[0][:]])
            # ---- chunks 1, 2 quant interleaved with |x*gam| absmax ----------
            for ci in (1, 2):
                for kt in range(KT):
                    w1_chunk_quant(ci, kt)
                    if kt % 2 == ci - 1:
                        k2a = (ci - 1) * 8 + kt // 2
                        nc.scalar.activation(xga[:], xts[k2a][:], AF.Abs,
                                             bias=zero_col[:],
                                             scale=gam[:, k2a:k2a + 1])
                        nc.vector.tensor_tensor(acc[:], acc[:], xga[:], op=OP.max)
                nc.gpsimd.collective_compute(
                    "AllGather", OP.bypass, replica_groups=rg,
                    ins=[t1_store[ci][:]], outs=[t1_g[ci][:]])

            # ---- token rows: rstd + absmax -> sx, rsx -----------------------
            v_row = rring.tile([1, TOK], F32, tag="row")
            nc.vector.tensor_scalar(v_row[:, 0:512], ss_ps0[:], 1.0 / DIM, EPS,
                                    op0=OP.mult, op1=OP.add)
            nc.vector.tensor_scalar(v_row[:, 512:1024], ss_ps1[:], 1.0 / DIM, EPS,
                                    op0=OP.mult, op1=OP.add)
            sq_row = rring.tile([1, TOK], F32, tag="row")
            nc.scalar.activation(sq_row[:], v_row[:], AF.Sqrt, bias=zero_col[0:1, :])
            rstd_row = rring.tile([1, TOK], F32, tag="row")
            nc.vector.reciprocal_approx_accurate(rstd_row[:], sq_row[:], rscr[:])

            for c in range(8):
                pt = ps_tr.tile([128, 128], F32, tag="tr")
                nc.tensor.transpose(pt[:], acc[:, c * 128:(c + 1) * 128], ident[:])
                nc.vector.tensor_reduce(m0t[:, c:c + 1], pt[:], axis=AX.X, op=OP.max)
            nc.gpsimd.dma_start(r1_d[:].rearrange("c p -> p c"), m0t[:])
            m0row = rring.tile([1, TOK], F32, tag="row")
            nc.gpsimd.dma_start(m0row[:], r1_d[:].rearrange("c p -> (c p)")[None, :])
            nc.vector.tensor_tensor(m0row[:], m0row[:], rstd_row[:], op=OP.mult)
            nc.vector.tensor_scalar(m0row[:], m0row[:], 1e-5, None, op0=OP.max)
            sx_row = rring.tile([1, TOK], F32, tag="row")
            nc.vector.reciprocal_approx_accurate(sx_row[:], m0row[:], rscr[:])
            nc.vector.tensor_scalar(sx_row[:], sx_row[:], 127.0, None, op0=OP.mult)
            nc.vector.reciprocal_approx_accurate(inv_sx[:], sx_row[:], rscr[:])
            nc.vector.tensor_tensor(rstd_row[:], rstd_row[:], sx_row[:], op=OP.mult)
            bcast_row(rsx, rstd_row, TOK)

            # ---- quantize x: n_xT = round((x*gam)*rsx) ----------------------
            for kt in range(KT):
                xr = big.tile([128, TOK], F32, tag="big")
                nc.sync.dma_start(xr[:], xT[kt * 128:(kt + 1) * 128, :])
                t = fsc.tile([128, TOK], F32, tag="fsc")
                nc.scalar.activation(t[:], xr[:], AF.Copy, bias=0.0,
                                     scale=gam[:, kt:kt + 1])
                nc.vector.tensor_tensor(t[:], t[:], rsx[:], op=OP.mult)
                nc.vector.tensor_scalar(nxT[:, kt * TOK:(kt + 1) * TOK], t[:], MAGIC, MAGIC,
                                        op0=OP.add, op1=OP.subtract)

            # ---- s1 row: invw1 * inv_sx -------------------------------------
            s1_row = rring.tile([1, TOK], F32, tag="row")
            nc.vector.tensor_scalar(s1_row[:], inv_sx[:], invw[:, 0:1], None, op0=OP.mult)
            bcast_row(s1r, s1_row, TOK)
            # reset absmax accumulator for the h phase (accA | accB halves)
            nc.vector.memset(acc[:], 0.0)

            # ---- w2 quant: loads on gpsimd, scalar round, vec clip ----------
            for ht in range(HBL):
                for hf in range(2):
                    w2l = wio2.tile([128, TOK], F32, tag="w2")
                    nc.gpsimd.dma_start(w2l[:], w2s[ht * 128:(ht + 1) * 128,
                                                    hf * 1024:(hf + 1) * 1024])
                    wf2 = fsc.tile([128, TOK], F32, tag="fsc")
                    nc.scalar.activation(wf2[:], w2l[:], AF.Copy,
                                         bias=MAGIC, scale=swb[:, 1:2])
                    q2 = qio2.tile([128, TOK], BF16, tag="q2")
                    nc.scalar.activation(q2[:], wf2[:], AF.Copy, bias=-MAGIC)
                    nc.vector.tensor_scalar(q2[:], q2[:], 1.0, -1.0,
                                            op0=OP.min, op1=OP.max)
                    d0 = hf * 8
                    nc.gpsimd.dma_start(
                        t2_store[d0:d0 + 8, :, ht * 128:(ht + 1) * 128].rearrange(
                            "d k j -> k d j"),
                        q2[:].rearrange("k (d j) -> k d j", d=8))
            # pin: last write into t2_store is data-dependent on t1_g[2] (AG2
            # output), so the w2 AllGather cannot be scheduled before w1's AGs.
            nc.gpsimd.dma_start(pin_t[:], t2_store[0, 0:2, 0:1])
            nc.gpsimd.dma_start(pin_d[:], t1_g[NAG - 1][0, 0, 0:2, 0:1])
            nc.gpsimd.tensor_scalar(pin_zf[:], pin_d[:], 0.0, None, op0=OP.mult)
            nc.gpsimd.tensor_tensor(pin_z[:], pin_zf[:], pin_t[:], op=OP.add)
            nc.gpsimd.dma_start(t2_store[0, 0:2, 0:1], pin_z[:])
            nc.gpsimd.collective_compute(
                "AllGather", OP.bypass, replica_groups=rg, ins=[t2_store[:]], outs=[t2_g[:]])

            # ============ MM1, token half A (all 64 hid blocks) ==============
            # blocks processed in PAIRS with two PSUM banks alternating per
            # matmul instruction: back-to-back accumulation into one bank
            # stalls the PE pipe (~400ns/mm vs ~190ns with alternation).
            blocks = [(ci, r, bi) for ci in range(NAG) for r in range(n_cores)
                      for bi in range(CHUNKS[ci])]
            pairs = [(blocks[2 * i], blocks[2 * i + 1]) for i in range(len(blocks) // 2)]

            def mm1_pair(blkA, blkB, th, htile):
                to = th * 512
                wbt = []
                for (ci, r, bi) in (blkA, blkB):
                    wb = pw.tile([128, KT * 128], FP8, tag="wb")
                    nc.sync.dma_start(wb[:], t1_g[ci][r, bi])
                    wbt.append(wb)
                psA = ps_mm.tile([128, 512], F32, tag="mm")
                psB = ps_mm.tile([128, 512], F32, tag="mm")
                for kt in range(KT):
                    st, sp = (kt == 0), (kt == KT - 1)
                    mv = nxT[:, kt * TOK + to:kt * TOK + to + 512]
                    nc.tensor.matmul(psA[:], wbt[0][:, kt * 128:(kt + 1) * 128], mv,
                                     start=st, stop=sp)
                    nc.tensor.matmul(psB[:], wbt[1][:, kt * 128:(kt + 1) * 128], mv,
                                     start=st, stop=sp)
                for (ci, r, bi), ps in ((blkA, psA), (blkB, psB)):
                    ghb = r * HBL + OFFS[ci] + bi
                    hs = psc.tile([128, 512], F32, tag="hs")
                    nc.vector.tensor_tensor(hs[:], ps[:], s1r[:, to:to + 512], op=OP.mult)
                    hsl = htile[:, ghb * 512:(ghb + 1) * 512]
                    nc.scalar.activation(hsl, hs[:], AF.Gelu, bias=zero_col[:])
                    ga = psc.tile([128, 512], BF16, tag="ga")
                    nc.scalar.activation(ga[:], hsl, AF.Abs, bias=zero_col[:])
                    nc.vector.tensor_tensor(acc[:, to:to + 512], acc[:, to:to + 512],
                                            ga[:], op=OP.max)

            for (blkA, blkB) in pairs:
                mm1_pair(blkA, blkB, 0, h0)

        # ---- phase A scratch pool closed; h1 + late rows live in its space --
        with ExitStack() as sb:
            hp1 = sb.enter_context(tc.tile_pool(name="hp1", bufs=1))
            rowp = sb.enter_context(tc.tile_pool(name="rowp", bufs=1))
            h1 = hp1.tile([128, HB * 512], BF16)
            # vector-only scratch rows, both on partition 0 (engine AP rule);
            # the s2/i2 rows themselves are computed into partition 0 of their
            # broadcast tiles.
            lrows = rowp.tile([1, 1024], F32)

            def half_rows(th):
                """acc[:, th*512:+512] absmax -> s2row/i2row for that half."""
                to = th * 512
                for c in range(4):
                    pt = ps_tr.tile([128, 128], F32, tag="tr")
                    nc.tensor.transpose(pt[:], acc[:, to + c * 128:to + (c + 1) * 128],
                                        ident[:])
                    nc.vector.tensor_reduce(m2t[:, c:c + 1], pt[:], axis=AX.X, op=OP.max)
                nc.sync.dma_start(r2_d[th][:].rearrange("c p -> p c"), m2t[:])
                m2row = lrows[0:1, 0:512]
                nc.sync.dma_start(m2row, r2_d[th][:].rearrange("c p -> (c p)")[None, :])
                nc.vector.tensor_scalar(m2row, m2row, 1e-5, None, op0=OP.max)
                rs2 = lrows[0:1, 512:1024]
                s2row = s2r[0:1, to:to + 512]
                nc.vector.reciprocal_approx_accurate(s2row, m2row, rs2)
                nc.vector.tensor_scalar(s2row, s2row, 127.0, None, op0=OP.mult)
                i2row = i2r[0:1, to:to + 512]
                nc.vector.reciprocal_approx_accurate(i2row, s2row, rs2)
                nc.vector.tensor_scalar(i2row, i2row, invw[:, 1:2], None, op0=OP.mult)
                return s2row, i2row

            def half_bcast(th, s2row, i2row):
                to = th * 512
                bcast_row(s2r, s2row, 512, off=to)
                bcast_row(i2r, i2row, 512, off=to)

            def q2_block(th, htile, kg):
                """requant h in place: vector scale-mult, scalar MAGIC round."""
                to = th * 512
                hsl = htile[:, kg * 512:(kg + 1) * 512]
                t2s = psc.tile([128, 512], F32, tag="hs")
                nc.vector.tensor_tensor(t2s[:], hsl, s2r[:, to:to + 512], op=OP.mult)
                rq = psc.tile([128, 512], F32, tag="hs")
                nc.scalar.activation(rq[:], t2s[:], AF.Copy, bias=MAGIC)
                nc.scalar.activation(hsl, rq[:], AF.Copy, bias=-MAGIC)

            # ============ MM1 half B, with rows(A) + Q2(A) hidden under it ===
            q2a = 0
            rowsA = [None]
            for idx, (blkA, blkB) in enumerate(pairs):
                mm1_pair(blkA, blkB, 1, h1)
                if idx == 2:
                    rowsA[0] = half_rows(0)
                elif idx == 4:
                    half_bcast(0, *rowsA[0])
                elif idx >= 5 and q2a < HB:
                    for _ in range(3):
                        if q2a < HB:
                            q2_block(0, h0, q2a)
                            q2a += 1
            while q2a < HB:
                q2_block(0, h0, q2a)
                q2a += 1

            # ============ MM2 per half, d-pairs with alternating PSUM banks ==
            def mm2_dpair(th, htile, d0):
                to = th * 512
                psA = ps_mm.tile([128, 512], F32, tag="mm")
                psB = ps_mm.tile([128, 512], F32, tag="mm")
                for r in range(n_cores):
                    wv0 = pmm2w.tile([128, HBL * 128], BF16, tag="wv")
                    nc.scalar.dma_start(wv0[:], t2_g[r, d0])
                    wv1 = pmm2w.tile([128, HBL * 128], BF16, tag="wv")
                    nc.sync.dma_start(wv1[:], t2_g[r, d0 + 1])
                    for k2 in range(HBL):
                        kg = r * HBL + k2
                        st, sp = (kg == 0), (kg == HB - 1)
                        mv = htile[:, kg * 512:(kg + 1) * 512]
                        nc.tensor.matmul(psA[:], wv0[:, k2 * 128:(k2 + 1) * 128], mv,
                                         start=st, stop=sp)
                        nc.tensor.matmul(psB[:], wv1[:, k2 * 128:(k2 + 1) * 128], mv,
                                         start=st, stop=sp)
                for d, ps in ((d0, psA), (d0 + 1, psB)):
                    ot = psc.tile([128, 512], F32, tag="hs")
                    nc.vector.tensor_tensor(ot[:], ps[:], i2r[:, to:to + 512], op=OP.mult)
                    nc.gpsimd.dma_start(outT[d * 128:(d + 1) * 128, to:to + 512], ot[:])

            q2b = 0
            rowsB = [None]
            for dp in range(DB // 2):
                mm2_dpair(0, h0, 2 * dp)
                if dp == 0:
                    rowsB[0] = half_rows(1)
                elif dp == 1:
                    half_bcast(1, *rowsB[0])
                elif q2b < HB:
                    for _ in range(11):
                        if q2b < HB:
                            q2_block(1, h1, q2b)
                            q2b += 1
            while q2b < HB:
                q2_block(1, h1, q2b)
                q2b += 1

            for dp in range(DB // 2):
                mm2_dpair(1, h1, 2 * dp)

    nc.compile()
    return nc


def _get_nc():
    if "nc" not in _cache:
        _cache["nc"] = _build()
    return _cache["nc"]


def _prep_inputs(x, w1, w2, gamma):
    x2d = np.ascontiguousarray(np.asarray(x, dtype=np.float32).reshape(NTOK, DIM))
    w1 = np.asarray(w1, dtype=np.float32)
    w2 = np.asarray(w2, dtype=np.float32)
    gamma = np.asarray(gamma, dtype=np.float32)
    w1T = np.ascontiguousarray(w1.T)          # [DIM, HID]
    w2T = np.ascontiguousarray(w2.T)          # [HID, DIM]
    gpt = np.ascontiguousarray(gamma.reshape(KT, 128).T)
    hs = HID // NCORES
    in_maps = []
    for c in range(NCORES):
        in_maps.append({
            "xT": np.ascontiguousarray(x2d[c * TOK:(c + 1) * TOK, :].T),
            "w1s": np.ascontiguousarray(w1T[:, c * hs:(c + 1) * hs]),
            "w2s": np.ascontiguousarray(w2T[c * hs:(c + 1) * hs, :]),
            "gpt": gpt,
        })
    return in_maps


def _run(in_maps, trace=False, **kw):
    nc = _get_nc()
    return bass_utils.run_bass_kernel_spmd(
        nc, in_maps, core_ids=list(range(NCORES)), trace=trace, **kw)


def kernel(x, w1, w2, gamma):
    in_maps = _prep_inputs(x, w1, w2, gamma)
    res = _run(in_maps, trace=False)
    out = np.empty((NTOK, DIM), dtype=np.float32)
    for c in range(NCORES):
        out[c * TOK:(c + 1) * TOK, :] = res.results[c]["outT"].T
    return out.reshape(B, S, DIM)


# revision 50
# speedup vs baseline: 1.2267x; 1.1550x over previous
"""BitMLP (BitNet-style MLP) Trainium2 kernel, 8-way data-parallel over tokens.

reference semantics:
  h   = act_quant(rms_norm(x, gamma)) @ w1q.T   (w1q = per-tensor ternary quant)
  out = act_quant(gelu_exact(h)) @ w2q.T

Key facts exploited:
  * act_quant produces n/scale with n an integer in [-127, 127]  -> n is exact in bf16
  * weight quant produces t*inv_w with t ternary in {-1, 0, 1}   -> t is exact in bf16
  * therefore both matmuls are exact integer accumulations computed in bf16 at
    full TensorE rate; per-token/per-tensor scales are applied afterwards.

Sharding (8 cores on one chip):
  * tokens (4*2048 = 8192) split 1024/core; each core computes its tokens' output
  * weight quantization is cooperative: core c quantizes 1/8 of w1 and w2,
    the per-tensor mean(|w|) is combined with a scalar AllReduce, and the
    ternary bf16 weights are AllGathered.

Final schedule (2212us baseline -> 1503us), the key lessons paid for in traces:
  * every engine queue is IN-ORDER: a semaphore wait at the head blocks the
    whole queue, so emission order per engine is scheduled explicitly.
    Collective triggers, w2-quant loads/stores and out-stores live on gpsimd;
    stats + chunk + wb/wv loads split across sync/scalar/gpsimd DMA queues.
  * phase A: w1/w2/x stats stream concurrently on three queues; the 2-scalar
    AllReduce (per-tensor mean|w|) triggers ~84us in; w1 ternarization is
    2 scalar activation ops (Copy w*s+MAGIC, Copy -MAGIC) + 1 vector clip,
    chunked [4,2,2] hid-blocks/core and AllGathered so MM1 starts while
    chunks 1,2 are still gathering; the w2 AllGather is pinned after w1's
    via a data dep through t1_g[2].
  * MM1 token-half-outer (A then B): h(A) absmax rows + requant Q2(A) hide
    under MM1(B), Q2(B) under MM2(A); no PE stall at phase boundaries.
    MM2 d-pair-outer per half; weight streams alternate scalar/sync queues.
  * absmax rows round-trips (m2 -> DRAM -> row) go on the SYNC queue so they
    cannot be delayed by the w2-AG pin chain on gpsimd.
  * remaining gap to the ~900us PE roofline is mostly DVFS throttling
    (throttle_active ~240-1000us depending on schedule density) plus the
    serial CC stream (AR + 3 AGs + w2 AG ~ 370us, partly hidden).
"""

import os
import sys

for _p in ("/root/.axon_site/_ro/trn_rl_repo", "/opt/trn_rl_repo"):
    if os.path.isdir(_p) and _p not in sys.path:
        sys.path.append(_p)

from contextlib import ExitStack

import numpy as np

from concourse import bacc, bass, masks, mybir, tile
from concourse import bass_utils

F32 = mybir.dt.float32
BF16 = mybir.dt.bfloat16
FP8 = mybir.dt.float8e4    # e4m3: ternary {-1,0,1} exact
AF = mybir.ActivationFunctionType
OP = mybir.AluOpType
AX = mybir.AxisListType

NCORES = 8
B, S, DIM, HID = 4, 2048, 2048, 8192
NTOK = B * S            # 8192
TOK = NTOK // NCORES    # 1024 tokens per core
KT = DIM // 128         # 16 k-tiles
HB = HID // 128         # 64 hid blocks
DB = DIM // 128         # 16 dim blocks
HBL = HID // NCORES // 128  # 8 hid blocks owned per core
CHUNKS = [4, 2, 2]      # w1 AG chunk sizes (hid blocks per core)
OFFS = [0, 4, 6]
NAG = len(CHUNKS)
MAGIC = 12582912.0      # 1.5 * 2**23: (v + MAGIC) - MAGIC == round-half-even(v)
EPS = 1e-6
W_NELEM = float(DIM * HID)

_cache = {}


def _build(n_cores=NCORES):
    nc = bacc.Bacc("TRN2", target_bir_lowering=False, debug=False, num_devices=n_cores)
    xT = nc.dram_tensor("xT", [DIM, TOK], F32, kind="ExternalInput")
    w1s = nc.dram_tensor("w1s", [DIM, HID // n_cores], F32, kind="ExternalInput")
    w2s = nc.dram_tensor("w2s", [HID // n_cores, DIM], F32, kind="ExternalInput")
    gpt = nc.dram_tensor("gpt", [128, KT], F32, kind="ExternalInput")
    outT = nc.dram_tensor("outT", [DIM, TOK], F32, kind="ExternalOutput")
    rg = [list(range(n_cores))]

    with tile.TileContext(nc) as tc, ExitStack() as ctx:
        misc = ctx.enter_context(tc.tile_pool(name="misc", bufs=1))
        xq = ctx.enter_context(tc.tile_pool(name="xq", bufs=1))
        hp0 = ctx.enter_context(tc.tile_pool(name="hp0", bufs=1))
        pw = ctx.enter_context(tc.tile_pool(name="pw", bufs=3))
        psc = ctx.enter_context(tc.tile_pool(name="psc", bufs=2))
        pmm2w = ctx.enter_context(tc.tile_pool(name="pmm2w", bufs=4))
        ps_mm = ctx.enter_context(tc.tile_pool(name="ps_mm", bufs=4, space="PSUM"))
        ps_tr = ctx.enter_context(tc.tile_pool(name="ps_tr", bufs=2, space="PSUM"))
        ps_ss = ctx.enter_context(tc.tile_pool(name="ps_ss", bufs=1, space="PSUM"))
        dram = ctx.enter_context(tc.tile_pool(name="dram", bufs=1, space="DRAM"))

        ident = misc.tile([128, 128], F32)
        masks.make_identity(nc, ident[:])
        zero_col = misc.tile([128, 1], F32)
        nc.vector.memset(zero_col[:], 0.0)
        ones_row = misc.tile([1, 128], F32)
        nc.vector.memset(ones_row[:], 1.0)
        ones_bf = misc.tile([128, 1], BF16)
        nc.vector.memset(ones_bf[:], 1.0)
        ones_f = misc.tile([128, 1], F32)
        nc.vector.memset(ones_f[:], 1.0)
        # persistent scale rows / broadcast tiles
        s1r = misc.tile([128, TOK], F32)        # (invw1 * inv_sx) per token
        s2r = misc.tile([128, TOK], F32)        # s2 per token, both halves
        i2r = misc.tile([128, TOK], F32)        # invw2 * inv_s2 per token
        invw = misc.tile([1, 2], F32)
        swb = misc.tile([128, 2], F32)
        gam = misc.tile([128, KT], F32)
        acc = misc.tile([128, TOK], F32)        # absmax accumulator (reused per phase)
        S1c = misc.tile([128, 2 * KT], F32)
        S2c = misc.tile([128, KT], F32)
        S12 = misc.tile([128, 2], F32)
        tot_sb = misc.tile([2, 1], F32)
        m0t = misc.tile([128, 8], F32)
        m2t = misc.tile([128, 4], F32)
        pin_t = misc.tile([2, 1], FP8)
        pin_d = misc.tile([2, 1], FP8)
        pin_zf = misc.tile([2, 1], F32)
        pin_z = misc.tile([2, 1], FP8)

        def bcast_row(dst, src_row, n, off=0):
            """dst[128, off:off+n] = broadcast of src_row[1, n] via PE outer product."""
            for o in range(0, n, 512):
                w = min(512, n - o)
                ps = ps_mm.tile([128, 512], F32, tag="mm")
                nc.tensor.matmul(ps[:, 0:w], ones_row[:], src_row[:, o:o + w],
                                 start=True, stop=True)
                nc.scalar.activation(dst[:, off + o:off + o + w], ps[:, 0:w], AF.Copy, bias=0.0)

        # DRAM scratch
        ar_in = dram.tile([2, 1], F32)
        ar_out = dram.tile([2, 1], F32, addr_space="Shared")
        t1_store = [dram.tile([CHUNKS[i], 128, KT * 128], FP8, name=f"t1s{i}")
                    for i in range(NAG)]
        t1_g = [dram.tile([n_cores, CHUNKS[i], 128, KT * 128], FP8, addr_space="Shared",
                          name=f"t1g{i}") for i in range(NAG)]
        t2_store = dram.tile([DB, 128, HBL * 128], FP8)
        t2_g = dram.tile([n_cores, DB, 128, HBL * 128], FP8, addr_space="Shared")
        r1_d = dram.tile([8, 128], F32)
        r2_d = [dram.tile([4, 128], F32, name=f"r2d{t}") for t in range(2)]

        nc.sync.dma_start(gam[:], gpt[:])

        nxT = xq.tile([128, KT * TOK], BF16)
        h0 = hp0.tile([128, HB * 512], BF16)

        with ExitStack() as sa:
            big = sa.enter_context(tc.tile_pool(name="big", bufs=2))     # w2/x/xre
            wio2 = sa.enter_context(tc.tile_pool(name="wio2", bufs=2))   # w1 stats + w2q loads
            scx2 = sa.enter_context(tc.tile_pool(name="scx2", bufs=1))
            xgap = sa.enter_context(tc.tile_pool(name="xgap", bufs=1))
            fsc = sa.enter_context(tc.tile_pool(name="fsc", bufs=2))     # f32 scratch ring
            wio = sa.enter_context(tc.tile_pool(name="wio", bufs=3))     # w1 chunk col loads
            qio = sa.enter_context(tc.tile_pool(name="qio", bufs=2))     # w1 ternary bf16
            qio2 = sa.enter_context(tc.tile_pool(name="qio2", bufs=1))   # w2 ternary bf16
            rring = sa.enter_context(tc.tile_pool(name="rring", bufs=3))
            sax = sa.enter_context(tc.tile_pool(name="sax", bufs=1))

            rsx = sax.tile([128, TOK], F32)
            inv_sx = sax.tile([1, TOK], F32)
            rscr = sax.tile([1, TOK], F32)
            xga = xgap.tile([128, TOK], F32)

            nc.vector.memset(acc[:], 0.0)

            # ---- |w1| stats: full-row loads split across sync + gpsimd ------
            for kt in range(KT):
                wt = wio2.tile([128, TOK], F32, tag="w2")
                eng = nc.sync if kt % 2 == 0 else nc.gpsimd
                eng.dma_start(wt[:], w1s[kt * 128:(kt + 1) * 128, :])
                nc.vector.tensor_reduce(S1c[:, kt:kt + 1], wt[:], axis=AX.X, op=OP.add,
                                        apply_absolute_value=True)
            for ht in range(HBL):
                for hf in range(2):
                    w2t = big.tile([128, TOK], F32, tag="big")
                    nc.scalar.dma_start(w2t[:], w2s[ht * 128:(ht + 1) * 128,
                                                    hf * 1024:(hf + 1) * 1024])
                    nc.vector.tensor_reduce(S2c[:, 2 * ht + hf:2 * ht + hf + 1], w2t[:],
                                            axis=AX.X, op=OP.add,
                                            apply_absolute_value=True)
            nc.vector.tensor_reduce(S12[:, 0:1], S1c[:, 0:KT], axis=AX.X, op=OP.add)
            nc.vector.tensor_reduce(S12[:, 1:2], S2c[:], axis=AX.X, op=OP.add)
            tot_ps = ps_tr.tile([2, 1], F32, tag="tr")
            nc.tensor.matmul(tot_ps[:], S12[:], ones_f[:], start=True, stop=True)
            nc.vector.tensor_copy(tot_sb[:], tot_ps[:])
            nc.gpsimd.dma_start(ar_in[:], tot_sb[:])
            nc.gpsimd.collective_compute(
                "AllReduce", OP.add, replica_groups=rg, ins=[ar_in[:]], outs=[ar_out[:]])

            # ---- AllReduce result -> weight scales (gpsimd queue: the sync
            # queue must not stall on AR completion ahead of the chunk loads) -
            tot2 = rring.tile([1, TOK], F32, tag="row")
            nc.gpsimd.dma_start(tot2[:, 0:2], ar_out[:].rearrange("a b -> b a"))
            nc.vector.tensor_scalar(invw[:], tot2[:, 0:2], 1.0 / W_NELEM, 1e-5,
                                    op0=OP.mult, op1=OP.max)
            sw = rring.tile([1, TOK], F32, tag="row")
            nc.vector.reciprocal(sw[:, 0:2], invw[:])
            ps_b = ps_tr.tile([128, 2], F32, tag="tr")
            nc.tensor.matmul(ps_b[:], ones_row[:], sw[:, 0:2], start=True, stop=True)
            nc.scalar.activation(swb[:], ps_b[:], AF.Copy, bias=0.0)

            def w1_chunk_quant(ci, kt):
                CH = CHUNKS[ci]
                CW = CH * 128
                wq = wio.tile([128, 4 * 128], F32, tag="wq")
                nc.sync.dma_start(wq[:, 0:CW], w1s[kt * 128:(kt + 1) * 128,
                                                   OFFS[ci] * 128:OFFS[ci] * 128 + CW])
                wf = fsc.tile([128, TOK], F32, tag="fsc")
                nc.scalar.activation(wf[:, 0:CW], wq[:, 0:CW], AF.Copy,
                                     bias=MAGIC, scale=swb[:, 0:1])
                # clip in the MAGIC domain (f32), then write the ternary as fp8
                nc.vector.tensor_scalar(wf[:, 0:CW], wf[:, 0:CW],
                                        MAGIC + 1.0, MAGIC - 1.0,
                                        op0=OP.min, op1=OP.max)
                q = qio.tile([128, 4 * 128], FP8, tag="q")
                nc.scalar.activation(q[:, 0:CW], wf[:, 0:CW], AF.Copy, bias=-MAGIC)
                nc.gpsimd.dma_start(
                    t1_store[ci][:, :, kt * 128:(kt + 1) * 128].rearrange(
                        "b k j -> k b j"),
                    q[:, 0:CW].rearrange("k (b j) -> k b j", b=CH))

            # ---- chunk 0 quant interleaved with x loads + sum-of-squares ----
            ss_ps0 = ps_ss.tile([1, 512], F32, tag="ss0")
            ss_ps1 = ps_ss.tile([1, 512], F32, tag="ss1")
            xts = []
            for kt in range(KT):
                w1_chunk_quant(0, kt)
                xt = big.tile([128, TOK], F32, tag="big")
                nc.gpsimd.dma_start(xt[:], xT[kt * 128:(kt + 1) * 128, :])
                xts.append(xt)
                x2 = scx2.tile([128, TOK], BF16, tag="x2")
                nc.vector.tensor_tensor(x2[:], xt[:], xt[:], op=OP.mult)
                nc.tensor.matmul(ss_ps0[:], ones_bf[:], x2[:, 0:512],
                                 start=(kt == 0), stop=(kt == KT - 1))
                nc.tensor.matmul(ss_ps1[:], ones_bf[:], x2[:, 512:1024],
                                 start=(kt == 0), stop=(kt == KT - 1))
            nc.gpsimd.collective_compute(
                "AllGather", OP.bypass, replica_groups=rg,
                ins=[t1_store[0][:]], outs=[t1_g[0][:]])
            # ---- chunks 1, 2 quant interleaved with |x*gam| absmax ----------
            for ci in (1, 2):
                for kt in range(KT):
                    w1_chunk_quant(ci, kt)
                    if kt % 2 == ci - 1:
                        k2a = (ci - 1) * 8 + kt // 2
                        nc.scalar.activation(xga[:], xts[k2a][:], AF.Abs,
                                             bias=zero_col[:],
                                             scale=gam[:, k2a:k2a + 1])
                        nc.vector.tensor_tensor(acc[:], acc[:], xga[:], op=OP.max)
                nc.gpsimd.collective_compute(
                    "AllGather", OP.bypass, replica_groups=rg,
                    ins=[t1_store[ci][:]], outs=[t1_g[ci][:]])

            # ---- token rows: rstd + absmax -> sx, rsx -----------------------
            v_row = rring.tile([1, TOK], F32, tag="row")
            nc.vector.tensor_scalar(v_row[:, 0:512], ss_ps0[:], 1.0 / DIM, EPS,
                                    op0=OP.mult, op1=OP.add)
            nc.vector.tensor_scalar(v_row[:, 512:1024], ss_ps1[:], 1.0 / DIM, EPS,
                                    op0=OP.mult, op1=OP.add)
            sq_row = rring.tile([1, TOK], F32, tag="row")
            nc.scalar.activation(sq_row[:], v_row[:], AF.Sqrt, bias=zero_col[0:1, :])
            rstd_row = rring.tile([1, TOK], F32, tag="row")
            nc.vector.reciprocal_approx_accurate(rstd_row[:], sq_row[:], rscr[:])

            for c in range(8):
                pt = ps_tr.tile([128, 128], F32, tag="tr")
                nc.tensor.transpose(pt[:], acc[:, c * 128:(c + 1) * 128], ident[:])
                nc.vector.tensor_reduce(m0t[:, c:c + 1], pt[:], axis=AX.X, op=OP.max)
            nc.gpsimd.dma_start(r1_d[:].rearrange("c p -> p c"), m0t[:])
            m0row = rring.tile([1, TOK], F32, tag="row")
            nc.gpsimd.dma_start(m0row[:], r1_d[:].rearrange("c p -> (c p)")[None, :])
            nc.vector.tensor_tensor(m0row[:], m0row[:], rstd_row[:], op=OP.mult)
            nc.vector.tensor_scalar(m0row[:], m0row[:], 1e-5, None, op0=OP.max)
            sx_row = rring.tile([1, TOK], F32, tag="row")
            nc.vector.reciprocal_approx_accurate(sx_row[:], m0row[:], rscr[:])
            nc.vector.tensor_scalar(sx_row[:], sx_row[:], 127.0, None, op0=OP.mult)
            nc.vector.reciprocal_approx_accurate(inv_sx[:], sx_row[:], rscr[:])
            nc.vector.tensor_tensor(rstd_row[:], rstd_row[:], sx_row[:], op=OP.mult)
            bcast_row(rsx, rstd_row, TOK)

            # ---- quantize x: n_xT = round((x*gam)*rsx) ----------------------
            for kt in range(KT):
                xr = big.tile([128, TOK], F32, tag="big")
                nc.sync.dma_start(xr[:], xT[kt * 128:(kt + 1) * 128, :])
                t = fsc.tile([128, TOK], F32, tag="fsc")
                nc.scalar.activation(t[:], xr[:], AF.Copy, bias=0.0,
                                     scale=gam[:, kt:kt + 1])
                nc.vector.tensor_tensor(t[:], t[:], rsx[:], op=OP.mult)
                nc.vector.tensor_scalar(nxT[:, kt * TOK:(kt + 1) * TOK], t[:], MAGIC, MAGIC,
                                        op0=OP.add, op1=OP.subtract)

            # ---- s1 row: invw1 * inv_sx -------------------------------------
            s1_row = rring.tile([1, TOK], F32, tag="row")
            nc.vector.tensor_scalar(s1_row[:], inv_sx[:], invw[:, 0:1], None, op0=OP.mult)
            bcast_row(s1r, s1_row, TOK)
            # reset absmax accumulator for the h phase (accA | accB halves)
            nc.vector.memset(acc[:], 0.0)

            # ---- w2 quant: loads on gpsimd, scalar round, vec clip ----------
            for ht in range(HBL):
                for hf in range(2):
                    w2l = wio2.tile([128, TOK], F32, tag="w2")
                    nc.gpsimd.dma_start(w2l[:], w2s[ht * 128:(ht + 1) * 128,
                                                    hf * 1024:(hf + 1) * 1024])
                    wf2 = fsc.tile([128, TOK], F32, tag="fsc")
                    nc.scalar.activation(wf2[:], w2l[:], AF.Copy,
                                         bias=MAGIC, scale=swb[:, 1:2])
                    nc.vector.tensor_scalar(wf2[:], wf2[:], MAGIC + 1.0, MAGIC - 1.0,
                                            op0=OP.min, op1=OP.max)
                    q2 = qio2.tile([128, TOK], FP8, tag="q2")
                    nc.scalar.activation(q2[:], wf2[:], AF.Copy, bias=-MAGIC)
                    d0 = hf * 8
                    nc.gpsimd.dma_start(
                        t2_store[d0:d0 + 8, :, ht * 128:(ht + 1) * 128].rearrange(
                            "d k j -> k d j"),
                        q2[:].rearrange("k (d j) -> k d j", d=8))
            # pin: last write into t2_store is data-dependent on t1_g[2] (AG2
            # output), so the w2 AllGather cannot be scheduled before w1's AGs.
            nc.gpsimd.dma_start(pin_t[:], t2_store[0, 0:2, 0:1])
            nc.gpsimd.dma_start(pin_d[:], t1_g[NAG - 1][0, 0, 0:2, 0:1])
            nc.gpsimd.tensor_scalar(pin_zf[:], pin_d[:], 0.0, None, op0=OP.mult)
            nc.gpsimd.tensor_tensor(pin_z[:], pin_zf[:], pin_t[:], op=OP.add)
            nc.gpsimd.dma_start(t2_store[0, 0:2, 0:1], pin_z[:])
            nc.gpsimd.collective_compute(
                "AllGather", OP.bypass, replica_groups=rg, ins=[t2_store[:]], outs=[t2_g[:]])

            # ============ MM1, token half A (all 64 hid blocks) ==============
            # blocks processed in PAIRS with two PSUM banks alternating per
            # matmul instruction: back-to-back accumulation into one bank
            # stalls the PE pipe (~400ns/mm vs ~190ns with alternation).
            blocks = [(ci, r, bi) for ci in range(NAG) for r in range(n_cores)
                      for bi in range(CHUNKS[ci])]
            pairs = [(blocks[2 * i], blocks[2 * i + 1]) for i in range(len(blocks) // 2)]

            def mm1_pair(blkA, blkB, th, htile):
                to = th * 512
                wbt = []
                for (ci, r, bi) in (blkA, blkB):
                    wb = pw.tile([128, KT * 128], FP8, tag="wb")
                    nc.sync.dma_start(wb[:], t1_g[ci][r, bi])
                    wbt.append(wb)
                psA = ps_mm.tile([128, 512], F32, tag="mm")
                psB = ps_mm.tile([128, 512], F32, tag="mm")
                for kt in range(KT):
                    st, sp = (kt == 0), (kt == KT - 1)
                    mv = nxT[:, kt * TOK + to:kt * TOK + to + 512]
                    nc.tensor.matmul(psA[:], wbt[0][:, kt * 128:(kt + 1) * 128], mv,
                                     start=st, stop=sp)
                    nc.tensor.matmul(psB[:], wbt[1][:, kt * 128:(kt + 1) * 128], mv,
                                     start=st, stop=sp)
                for (ci, r, bi), ps in ((blkA, psA), (blkB, psB)):
                    ghb = r * HBL + OFFS[ci] + bi
                    hs = psc.tile([128, 512], F32, tag="hs")
                    nc.vector.tensor_tensor(hs[:], ps[:], s1r[:, to:to + 512], op=OP.mult)
                    hsl = htile[:, ghb * 512:(ghb + 1) * 512]
                    nc.scalar.activation(hsl, hs[:], AF.Gelu, bias=zero_col[:])
                    ga = psc.tile([128, 512], BF16, tag="ga")
                    nc.scalar.activation(ga[:], hsl, AF.Abs, bias=zero_col[:])
                    nc.vector.tensor_tensor(acc[:, to:to + 512], acc[:, to:to + 512],
                                            ga[:], op=OP.max)

            for (blkA, blkB) in pairs:
                mm1_pair(blkA, blkB, 0, h0)

        # ---- phase A scratch pool closed; h1 + late rows live in its space --
        with ExitStack() as sb:
            hp1 = sb.enter_context(tc.tile_pool(name="hp1", bufs=1))
            rowp = sb.enter_context(tc.tile_pool(name="rowp", bufs=1))
            h1 = hp1.tile([128, HB * 512], BF16)
            # vector-only scratch rows, both on partition 0 (engine AP rule);
            # the s2/i2 rows themselves are computed into partition 0 of their
            # broadcast tiles.
            lrows = rowp.tile([1, 1024], F32)

            def half_rows(th):
                """acc[:, th*512:+512] absmax -> s2row/i2row for that half."""
                to = th * 512
                for c in range(4):
                    pt = ps_tr.tile([128, 128], F32, tag="tr")
                    nc.tensor.transpose(pt[:], acc[:, to + c * 128:to + (c + 1) * 128],
                                        ident[:])
                    nc.vector.tensor_reduce(m2t[:, c:c + 1], pt[:], axis=AX.X, op=OP.max)
                nc.sync.dma_start(r2_d[th][:].rearrange("c p -> p c"), m2t[:])
                m2row = lrows[0:1, 0:512]
                nc.sync.dma_start(m2row, r2_d[th][:].rearrange("c p -> (c p)")[None, :])
                nc.vector.tensor_scalar(m2row, m2row, 1e-5, None, op0=OP.max)
                rs2 = lrows[0:1, 512:1024]
                s2row = s2r[0:1, to:to + 512]
                nc.vector.reciprocal_approx_accurate(s2row, m2row, rs2)
                nc.vector.tensor_scalar(s2row, s2row, 127.0, None, op0=OP.mult)
                i2row = i2r[0:1, to:to + 512]
                nc.vector.reciprocal_approx_accurate(i2row, s2row, rs2)
                nc.vector.tensor_scalar(i2row, i2row, invw[:, 1:2], None, op0=OP.mult)
                return s2row, i2row

            def half_bcast(th, s2row, i2row):
                to = th * 512
                bcast_row(s2r, s2row, 512, off=to)
                bcast_row(i2r, i2row, 512, off=to)

            def q2_block(th, htile, kg):
                """requant h in place: vector scale-mult, scalar MAGIC round."""
                to = th * 512
                hsl = htile[:, kg * 512:(kg + 1) * 512]
                t2s = psc.tile([128, 512], F32, tag="hs")
                nc.vector.tensor_tensor(t2s[:], hsl, s2r[:, to:to + 512], op=OP.mult)
                rq = psc.tile([128, 512], F32, tag="hs")
                nc.scalar.activation(rq[:], t2s[:], AF.Copy, bias=MAGIC)
                nc.scalar.activation(hsl, rq[:], AF.Copy, bias=-MAGIC)

            # ============ MM1 half B, with rows(A) + Q2(A) hidden under it ===
            q2a = 0
            rowsA = [None]
            for idx, (blkA, blkB) in enumerate(pairs):
                mm1_pair(blkA, blkB, 1, h1)
                if idx == 2:
                    rowsA[0] = half_rows(0)
                elif idx == 4:
                    half_bcast(0, *rowsA[0])
                elif idx >= 5 and q2a < HB:
                    for _ in range(3):
                        if q2a < HB:
                            q2_block(0, h0, q2a)
                            q2a += 1
            while q2a < HB:
                q2_block(0, h0, q2a)
                q2a += 1

            # ============ MM2 per half, d-pairs with alternating PSUM banks ==
            def mm2_dpair(th, htile, d0):
                to = th * 512
                psA = ps_mm.tile([128, 512], F32, tag="mm")
                psB = ps_mm.tile([128, 512], F32, tag="mm")
                for r in range(n_cores):
                    wv0 = pmm2w.tile([128, HBL * 128], FP8, tag="wv")
                    nc.scalar.dma_start(wv0[:], t2_g[r, d0])
                    wv1 = pmm2w.tile([128, HBL * 128], FP8, tag="wv")
                    nc.sync.dma_start(wv1[:], t2_g[r, d0 + 1])
                    for k2 in range(HBL):
                        kg = r * HBL + k2
                        st, sp = (kg == 0), (kg == HB - 1)
                        mv = htile[:, kg * 512:(kg + 1) * 512]
                        nc.tensor.matmul(psA[:], wv0[:, k2 * 128:(k2 + 1) * 128], mv,
                                         start=st, stop=sp)
                        nc.tensor.matmul(psB[:], wv1[:, k2 * 128:(k2 + 1) * 128], mv,
                                         start=st, stop=sp)
                for d, ps in ((d0, psA), (d0 + 1, psB)):
                    ot = psc.tile([128, 512], F32, tag="hs")
                    nc.vector.tensor_tensor(ot[:], ps[:], i2r[:, to:to + 512], op=OP.mult)
                    nc.gpsimd.dma_start(outT[d * 128:(d + 1) * 128, to:to + 512], ot[:])

            q2b = 0
            rowsB = [None]
            for dp in range(DB // 2):
                mm2_dpair(0, h0, 2 * dp)
                if dp == 0:
                    rowsB[0] = half_rows(1)
                elif dp == 1:
                    half_bcast(1, *rowsB[0])
                elif q2b < HB:
                    for _ in range(11):
                        if q2b < HB:
                            q2_block(1, h1, q2b)
                            q2b += 1
            while q2b < HB:
                q2_block(1, h1, q2b)
                q2b += 1

            for dp in range(DB // 2):
                mm2_dpair(1, h1, 2 * dp)

    nc.compile()
    return nc


def _get_nc():
    if "nc" not in _cache:
        _cache["nc"] = _build()
    return _cache["nc"]


def _prep_inputs(x, w1, w2, gamma):
    x2d = np.ascontiguousarray(np.asarray(x, dtype=np.float32).reshape(NTOK, DIM))
    w1 = np.asarray(w1, dtype=np.float32)
    w2 = np.asarray(w2, dtype=np.float32)
    gamma = np.asarray(gamma, dtype=np.float32)
    w1T = np.ascontiguousarray(w1.T)          # [DIM, HID]
    w2T = np.ascontiguousarray(w2.T)          # [HID, DIM]
    gpt = np.ascontiguousarray(gamma.reshape(KT, 128).T)
    hs = HID // NCORES
    in_maps = []
    for c in range(NCORES):
        in_maps.append({
            "xT": np.ascontiguousarray(x2d[c * TOK:(c + 1) * TOK, :].T),
            "w1s": np.ascontiguousarray(w1T[:, c * hs:(c + 1) * hs]),
            "w2s": np.ascontiguousarray(w2T[c * hs:(c + 1) * hs, :]),
            "gpt": gpt,
        })
    return in_maps


def _run(in_maps, trace=False, **kw):
    nc = _get_nc()
    return bass_utils.run_bass_kernel_spmd(
        nc, in_maps, core_ids=list(range(NCORES)), trace=trace, **kw)


def kernel(x, w1, w2, gamma):
    in_maps = _prep_inputs(x, w1, w2, gamma)
    res = _run(in_maps, trace=False)
    out = np.empty((NTOK, DIM), dtype=np.float32)
    for c in range(NCORES):
        out[c * TOK:(c + 1) * TOK, :] = res.results[c]["outT"].T
    return out.reshape(B, S, DIM)


# revision 52
# speedup vs baseline: 1.2841x; 1.0468x over previous
"""BitMLP (BitNet-style MLP) Trainium2 kernel, 8-way data-parallel over tokens.

reference semantics:
  h   = act_quant(rms_norm(x, gamma)) @ w1q.T   (w1q = per-tensor ternary quant)
  out = act_quant(gelu_exact(h)) @ w2q.T

Key facts exploited:
  * act_quant produces n/scale with n an integer in [-127, 127]  -> n is exact in bf16
  * weight quant produces t*inv_w with t ternary in {-1, 0, 1}   -> t is exact in bf16
  * therefore both matmuls are exact integer accumulations computed in bf16 at
    full TensorE rate; per-token/per-tensor scales are applied afterwards.

Sharding (8 cores on one chip):
  * tokens (4*2048 = 8192) split 1024/core; each core computes its tokens' output
  * weight quantization is cooperative: core c quantizes 1/8 of w1 and w2,
    the per-tensor mean(|w|) is combined with a scalar AllReduce, and the
    ternary bf16 weights are AllGathered.

Final schedule (2212us baseline -> 1503us), the key lessons paid for in traces:
  * every engine queue is IN-ORDER: a semaphore wait at the head blocks the
    whole queue, so emission order per engine is scheduled explicitly.
    Collective triggers, w2-quant loads/stores and out-stores live on gpsimd;
    stats + chunk + wb/wv loads split across sync/scalar/gpsimd DMA queues.
  * phase A: w1/w2/x stats stream concurrently on three queues; the 2-scalar
    AllReduce (per-tensor mean|w|) triggers ~84us in; w1 ternarization is
    2 scalar activation ops (Copy w*s+MAGIC, Copy -MAGIC) + 1 vector clip,
    chunked [4,2,2] hid-blocks/core and AllGathered so MM1 starts while
    chunks 1,2 are still gathering; the w2 AllGather is pinned after w1's
    via a data dep through t1_g[2].
  * MM1 token-half-outer (A then B): h(A) absmax rows + requant Q2(A) hide
    under MM1(B), Q2(B) under MM2(A); no PE stall at phase boundaries.
    MM2 d-pair-outer per half; weight streams alternate scalar/sync queues.
  * absmax rows round-trips (m2 -> DRAM -> row) go on the SYNC queue so they
    cannot be delayed by the w2-AG pin chain on gpsimd.
  * remaining gap to the ~900us PE roofline is mostly DVFS throttling
    (throttle_active ~240-1000us depending on schedule density) plus the
    serial CC stream (AR + 3 AGs + w2 AG ~ 370us, partly hidden).
"""

import os
import sys

for _p in ("/root/.axon_site/_ro/trn_rl_repo", "/opt/trn_rl_repo"):
    if os.path.isdir(_p) and _p not in sys.path:
        sys.path.append(_p)

from contextlib import ExitStack

import numpy as np

from concourse import bacc, bass, masks, mybir, tile
from concourse import bass_utils

F32 = mybir.dt.float32
BF16 = mybir.dt.bfloat16
FP8 = mybir.dt.float8e4    # e4m3: ternary {-1,0,1} exact
AF = mybir.ActivationFunctionType
OP = mybir.AluOpType
AX = mybir.AxisListType

NCORES = 8
B, S, DIM, HID = 4, 2048, 2048, 8192
NTOK = B * S            # 8192
TOK = NTOK // NCORES    # 1024 tokens per core
KT = DIM // 128         # 16 k-tiles
HB = HID // 128         # 64 hid blocks
DB = DIM // 128         # 16 dim blocks
HBL = HID // NCORES // 128  # 8 hid blocks owned per core
CHUNKS = [4, 2, 2]      # w1 AG chunk sizes (hid blocks per core)
OFFS = [0, 4, 6]
NAG = len(CHUNKS)
MAGIC = 12582912.0      # 1.5 * 2**23: (v + MAGIC) - MAGIC == round-half-even(v)
EPS = 1e-6
W_NELEM = float(DIM * HID)

_cache = {}


def _build(n_cores=NCORES):
    nc = bacc.Bacc("TRN2", target_bir_lowering=False, debug=False, num_devices=n_cores)
    xT = nc.dram_tensor("xT", [DIM, TOK], F32, kind="ExternalInput")
    w1s = nc.dram_tensor("w1s", [DIM, HID // n_cores], F32, kind="ExternalInput")
    w2s = nc.dram_tensor("w2s", [HID // n_cores, DIM], F32, kind="ExternalInput")
    gpt = nc.dram_tensor("gpt", [128, KT], F32, kind="ExternalInput")
    outT = nc.dram_tensor("outT", [DIM, TOK], F32, kind="ExternalOutput")
    rg = [list(range(n_cores))]

    with tile.TileContext(nc) as tc, ExitStack() as ctx:
        misc = ctx.enter_context(tc.tile_pool(name="misc", bufs=1))
        xq = ctx.enter_context(tc.tile_pool(name="xq", bufs=1))
        hp0 = ctx.enter_context(tc.tile_pool(name="hp0", bufs=1))
        pw = ctx.enter_context(tc.tile_pool(name="pw", bufs=6))
        psc = ctx.enter_context(tc.tile_pool(name="psc", bufs=2))
        pmm2w = ctx.enter_context(tc.tile_pool(name="pmm2w", bufs=6))
        ps_mm = ctx.enter_context(tc.tile_pool(name="ps_mm", bufs=4, space="PSUM"))
        ps_tr = ctx.enter_context(tc.tile_pool(name="ps_tr", bufs=2, space="PSUM"))
        ps_ss = ctx.enter_context(tc.tile_pool(name="ps_ss", bufs=1, space="PSUM"))
        dram = ctx.enter_context(tc.tile_pool(name="dram", bufs=1, space="DRAM"))

        ident = misc.tile([128, 128], F32)
        masks.make_identity(nc, ident[:])
        zero_col = misc.tile([128, 1], F32)
        nc.vector.memset(zero_col[:], 0.0)
        ones_row = misc.tile([1, 128], F32)
        nc.vector.memset(ones_row[:], 1.0)
        ones_bf = misc.tile([128, 1], BF16)
        nc.vector.memset(ones_bf[:], 1.0)
        ones_f = misc.tile([128, 1], F32)
        nc.vector.memset(ones_f[:], 1.0)
        # persistent scale rows / broadcast tiles
        s1r = misc.tile([128, TOK], F32)        # (invw1 * inv_sx) per token
        s2r = misc.tile([128, TOK], F32)        # s2 per token, both halves
        i2r = misc.tile([128, TOK], F32)        # invw2 * inv_s2 per token
        invw = misc.tile([1, 2], F32)
        swb = misc.tile([128, 2], F32)
        gam = misc.tile([128, KT], F32)
        acc = misc.tile([128, TOK], F32)        # absmax accumulator (reused per phase)
        S1c = misc.tile([128, 2 * KT], F32)
        S2c = misc.tile([128, KT], F32)
        S12 = misc.tile([128, 2], F32)
        tot_sb = misc.tile([2, 1], F32)
        m0t = misc.tile([128, 8], F32)
        m2t = misc.tile([128, 4], F32)
        pin_t = misc.tile([2, 1], FP8)
        pin_d = misc.tile([2, 1], FP8)
        pin_zf = misc.tile([2, 1], F32)
        pin_z = misc.tile([2, 1], FP8)

        def bcast_row(dst, src_row, n, off=0):
            """dst[128, off:off+n] = broadcast of src_row[1, n] via PE outer product."""
            for o in range(0, n, 512):
                w = min(512, n - o)
                ps = ps_mm.tile([128, 512], F32, tag="mm")
                nc.tensor.matmul(ps[:, 0:w], ones_row[:], src_row[:, o:o + w],
                                 start=True, stop=True)
                nc.scalar.activation(dst[:, off + o:off + o + w], ps[:, 0:w], AF.Copy, bias=0.0)

        # DRAM scratch
        ar_in = dram.tile([2, 1], F32)
        ar_out = dram.tile([2, 1], F32, addr_space="Shared")
        t1_store = [dram.tile([CHUNKS[i], 128, KT * 128], FP8, name=f"t1s{i}")
                    for i in range(NAG)]
        t1_g = [dram.tile([n_cores, CHUNKS[i], 128, KT * 128], FP8, addr_space="Shared",
                          name=f"t1g{i}") for i in range(NAG)]
        t2_store = dram.tile([DB, 128, HBL * 128], FP8)
        t2_g = dram.tile([n_cores, DB, 128, HBL * 128], FP8, addr_space="Shared")
        r1_d = dram.tile([8, 128], F32)
        r2_d = [dram.tile([4, 128], F32, name=f"r2d{t}") for t in range(2)]

        nc.sync.dma_start(gam[:], gpt[:])

        nxT = xq.tile([128, KT * TOK], BF16)
        h0 = hp0.tile([128, HB * 512], BF16)

        with ExitStack() as sa:
            big = sa.enter_context(tc.tile_pool(name="big", bufs=2))     # w2/x/xre
            wio2 = sa.enter_context(tc.tile_pool(name="wio2", bufs=2))   # w1 stats + w2q loads
            scx2 = sa.enter_context(tc.tile_pool(name="scx2", bufs=1))
            xgap = sa.enter_context(tc.tile_pool(name="xgap", bufs=1))
            fsc = sa.enter_context(tc.tile_pool(name="fsc", bufs=2))     # f32 scratch ring
            wio = sa.enter_context(tc.tile_pool(name="wio", bufs=3))     # w1 chunk col loads
            qio = sa.enter_context(tc.tile_pool(name="qio", bufs=2))     # w1 ternary bf16
            qio2 = sa.enter_context(tc.tile_pool(name="qio2", bufs=1))   # w2 ternary bf16
            rring = sa.enter_context(tc.tile_pool(name="rring", bufs=3))
            sax = sa.enter_context(tc.tile_pool(name="sax", bufs=1))

            rsx = sax.tile([128, TOK], F32)
            inv_sx = sax.tile([1, TOK], F32)
            rscr = sax.tile([1, TOK], F32)
            xga = xgap.tile([128, TOK], F32)

            nc.vector.memset(acc[:], 0.0)

            # ---- |w1| stats: full-row loads split across sync + gpsimd ------
            for kt in range(KT):
                wt = wio2.tile([128, TOK], F32, tag="w2")
                eng = nc.sync if kt % 2 == 0 else nc.gpsimd
                eng.dma_start(wt[:], w1s[kt * 128:(kt + 1) * 128, :])
                nc.vector.tensor_reduce(S1c[:, kt:kt + 1], wt[:], axis=AX.X, op=OP.add,
                                        apply_absolute_value=True)
            for ht in range(HBL):
                for hf in range(2):
                    w2t = big.tile([128, TOK], F32, tag="big")
                    nc.scalar.dma_start(w2t[:], w2s[ht * 128:(ht + 1) * 128,
                                                    hf * 1024:(hf + 1) * 1024])
                    nc.vector.tensor_reduce(S2c[:, 2 * ht + hf:2 * ht + hf + 1], w2t[:],
                                            axis=AX.X, op=OP.add,
                                            apply_absolute_value=True)
            nc.vector.tensor_reduce(S12[:, 0:1], S1c[:, 0:KT], axis=AX.X, op=OP.add)
            nc.vector.tensor_reduce(S12[:, 1:2], S2c[:], axis=AX.X, op=OP.add)
            tot_ps = ps_tr.tile([2, 1], F32, tag="tr")
            nc.tensor.matmul(tot_ps[:], S12[:], ones_f[:], start=True, stop=True)
            nc.vector.tensor_copy(tot_sb[:], tot_ps[:])
            nc.gpsimd.dma_start(ar_in[:], tot_sb[:])
            nc.gpsimd.collective_compute(
                "AllReduce", OP.add, replica_groups=rg, ins=[ar_in[:]], outs=[ar_out[:]])

            # ---- AllReduce result -> weight scales (gpsimd queue: the sync
            # queue must not stall on AR completion ahead of the chunk loads) -
            tot2 = rring.tile([1, TOK], F32, tag="row")
            nc.gpsimd.dma_start(tot2[:, 0:2], ar_out[:].rearrange("a b -> b a"))
            nc.vector.tensor_scalar(invw[:], tot2[:, 0:2], 1.0 / W_NELEM, 1e-5,
                                    op0=OP.mult, op1=OP.max)
            sw = rring.tile([1, TOK], F32, tag="row")
            nc.vector.reciprocal(sw[:, 0:2], invw[:])
            ps_b = ps_tr.tile([128, 2], F32, tag="tr")
            nc.tensor.matmul(ps_b[:], ones_row[:], sw[:, 0:2], start=True, stop=True)
            nc.scalar.activation(swb[:], ps_b[:], AF.Copy, bias=0.0)

            def w1_chunk_quant(ci, kt):
                CH = CHUNKS[ci]
                CW = CH * 128
                wq = wio.tile([128, 4 * 128], F32, tag="wq")
                nc.sync.dma_start(wq[:, 0:CW], w1s[kt * 128:(kt + 1) * 128,
                                                   OFFS[ci] * 128:OFFS[ci] * 128 + CW])
                wf = fsc.tile([128, TOK], F32, tag="fsc")
                nc.scalar.activation(wf[:, 0:CW], wq[:, 0:CW], AF.Copy,
                                     bias=MAGIC, scale=swb[:, 0:1])
                # clip in the MAGIC domain (f32), then write the ternary as fp8
                nc.vector.tensor_scalar(wf[:, 0:CW], wf[:, 0:CW],
                                        MAGIC + 1.0, MAGIC - 1.0,
                                        op0=OP.min, op1=OP.max)
                q = qio.tile([128, 4 * 128], FP8, tag="q")
                nc.scalar.activation(q[:, 0:CW], wf[:, 0:CW], AF.Copy, bias=-MAGIC)
                nc.gpsimd.dma_start(
                    t1_store[ci][:, :, kt * 128:(kt + 1) * 128].rearrange(
                        "b k j -> k b j"),
                    q[:, 0:CW].rearrange("k (b j) -> k b j", b=CH))

            # ---- chunk 0 quant interleaved with x loads + sum-of-squares ----
            ss_ps0 = ps_ss.tile([1, 512], F32, tag="ss0")
            ss_ps1 = ps_ss.tile([1, 512], F32, tag="ss1")
            xts = []
            for kt in range(KT):
                w1_chunk_quant(0, kt)
                xt = big.tile([128, TOK], F32, tag="big")
                nc.gpsimd.dma_start(xt[:], xT[kt * 128:(kt + 1) * 128, :])
                xts.append(xt)
                x2 = scx2.tile([128, TOK], BF16, tag="x2")
                nc.vector.tensor_tensor(x2[:], xt[:], xt[:], op=OP.mult)
                nc.tensor.matmul(ss_ps0[:], ones_bf[:], x2[:, 0:512],
                                 start=(kt == 0), stop=(kt == KT - 1))
                nc.tensor.matmul(ss_ps1[:], ones_bf[:], x2[:, 512:1024],
                                 start=(kt == 0), stop=(kt == KT - 1))
            nc.gpsimd.collective_compute(
                "AllGather", OP.bypass, replica_groups=rg,
                ins=[t1_store[0][:]], outs=[t1_g[0][:]])
            # ---- chunks 1, 2 quant interleaved with |x*gam| absmax ----------
            for ci in (1, 2):
                for kt in range(KT):
                    w1_chunk_quant(ci, kt)
                    if kt % 2 == ci - 1:
                        k2a = (ci - 1) * 8 + kt // 2
                        nc.scalar.activation(xga[:], xts[k2a][:], AF.Abs,
                                             bias=zero_col[:],
                                             scale=gam[:, k2a:k2a + 1])
                        nc.vector.tensor_tensor(acc[:], acc[:], xga[:], op=OP.max)
                nc.gpsimd.collective_compute(
                    "AllGather", OP.bypass, replica_groups=rg,
                    ins=[t1_store[ci][:]], outs=[t1_g[ci][:]])

            # ---- token rows: rstd + absmax -> sx, rsx -----------------------
            v_row = rring.tile([1, TOK], F32, tag="row")
            nc.vector.tensor_scalar(v_row[:, 0:512], ss_ps0[:], 1.0 / DIM, EPS,
                                    op0=OP.mult, op1=OP.add)
            nc.vector.tensor_scalar(v_row[:, 512:1024], ss_ps1[:], 1.0 / DIM, EPS,
                                    op0=OP.mult, op1=OP.add)
            sq_row = rring.tile([1, TOK], F32, tag="row")
            nc.scalar.activation(sq_row[:], v_row[:], AF.Sqrt, bias=zero_col[0:1, :])
            rstd_row = rring.tile([1, TOK], F32, tag="row")
            nc.vector.reciprocal_approx_accurate(rstd_row[:], sq_row[:], rscr[:])

            for c in range(8):
                pt = ps_tr.tile([128, 128], F32, tag="tr")
                nc.tensor.transpose(pt[:], acc[:, c * 128:(c + 1) * 128], ident[:])
                nc.vector.tensor_reduce(m0t[:, c:c + 1], pt[:], axis=AX.X, op=OP.max)
            nc.gpsimd.dma_start(r1_d[:].rearrange("c p -> p c"), m0t[:])
            m0row = rring.tile([1, TOK], F32, tag="row")
            nc.gpsimd.dma_start(m0row[:], r1_d[:].rearrange("c p -> (c p)")[None, :])
            nc.vector.tensor_tensor(m0row[:], m0row[:], rstd_row[:], op=OP.mult)
            nc.vector.tensor_scalar(m0row[:], m0row[:], 1e-5, None, op0=OP.max)
            sx_row = rring.tile([1, TOK], F32, tag="row")
            nc.vector.reciprocal_approx_accurate(sx_row[:], m0row[:], rscr[:])
            nc.vector.tensor_scalar(sx_row[:], sx_row[:], 127.0, None, op0=OP.mult)
            nc.vector.reciprocal_approx_accurate(inv_sx[:], sx_row[:], rscr[:])
            nc.vector.tensor_tensor(rstd_row[:], rstd_row[:], sx_row[:], op=OP.mult)
            bcast_row(rsx, rstd_row, TOK)

            # ---- quantize x: n_xT = round((x*gam)*rsx) ----------------------
            for kt in range(KT):
                xr = big.tile([128, TOK], F32, tag="big")
                nc.sync.dma_start(xr[:], xT[kt * 128:(kt + 1) * 128, :])
                t = fsc.tile([128, TOK], F32, tag="fsc")
                nc.scalar.activation(t[:], xr[:], AF.Copy, bias=0.0,
                                     scale=gam[:, kt:kt + 1])
                nc.vector.tensor_tensor(t[:], t[:], rsx[:], op=OP.mult)
                nc.vector.tensor_scalar(nxT[:, kt * TOK:(kt + 1) * TOK], t[:], MAGIC, MAGIC,
                                        op0=OP.add, op1=OP.subtract)

            # ---- s1 row: invw1 * inv_sx -------------------------------------
            s1_row = rring.tile([1, TOK], F32, tag="row")
            nc.vector.tensor_scalar(s1_row[:], inv_sx[:], invw[:, 0:1], None, op0=OP.mult)
            bcast_row(s1r, s1_row, TOK)
            # reset absmax accumulator for the h phase (accA | accB halves)
            nc.vector.memset(acc[:], 0.0)

            # ---- w2 quant: loads on gpsimd, scalar round, vec clip ----------
            for ht in range(HBL):
                for hf in range(2):
                    w2l = wio2.tile([128, TOK], F32, tag="w2")
                    nc.gpsimd.dma_start(w2l[:], w2s[ht * 128:(ht + 1) * 128,
                                                    hf * 1024:(hf + 1) * 1024])
                    wf2 = fsc.tile([128, TOK], F32, tag="fsc")
                    nc.scalar.activation(wf2[:], w2l[:], AF.Copy,
                                         bias=MAGIC, scale=swb[:, 1:2])
                    nc.vector.tensor_scalar(wf2[:], wf2[:], MAGIC + 1.0, MAGIC - 1.0,
                                            op0=OP.min, op1=OP.max)
                    q2 = qio2.tile([128, TOK], FP8, tag="q2")
                    nc.scalar.activation(q2[:], wf2[:], AF.Copy, bias=-MAGIC)
                    d0 = hf * 8
                    nc.gpsimd.dma_start(
                        t2_store[d0:d0 + 8, :, ht * 128:(ht + 1) * 128].rearrange(
                            "d k j -> k d j"),
                        q2[:].rearrange("k (d j) -> k d j", d=8))
            # pin: last write into t2_store is data-dependent on t1_g[2] (AG2
            # output), so the w2 AllGather cannot be scheduled before w1's AGs.
            nc.gpsimd.dma_start(pin_t[:], t2_store[0, 0:2, 0:1])
            nc.gpsimd.dma_start(pin_d[:], t1_g[NAG - 1][0, 0, 0:2, 0:1])
            nc.gpsimd.tensor_scalar(pin_zf[:], pin_d[:], 0.0, None, op0=OP.mult)
            nc.gpsimd.tensor_tensor(pin_z[:], pin_zf[:], pin_t[:], op=OP.add)
            nc.gpsimd.dma_start(t2_store[0, 0:2, 0:1], pin_z[:])
            nc.gpsimd.collective_compute(
                "AllGather", OP.bypass, replica_groups=rg, ins=[t2_store[:]], outs=[t2_g[:]])

            # ============ MM1, token half A (all 64 hid blocks) ==============
            # blocks processed in PAIRS with two PSUM banks alternating per
            # matmul instruction: back-to-back accumulation into one bank
            # stalls the PE pipe (~400ns/mm vs ~190ns with alternation).
            blocks = [(ci, r, bi) for ci in range(NAG) for r in range(n_cores)
                      for bi in range(CHUNKS[ci])]
            pairs = [(blocks[2 * i], blocks[2 * i + 1]) for i in range(len(blocks) // 2)]

            def mm1_pair(blkA, blkB, th, htile):
                to = th * 512
                wbt = []
                for (ci, r, bi) in (blkA, blkB):
                    wb = pw.tile([128, KT * 128], FP8, tag="wb")
                    nc.sync.dma_start(wb[:], t1_g[ci][r, bi])
                    wbt.append(wb)
                psA = ps_mm.tile([128, 512], F32, tag="mm")
                psB = ps_mm.tile([128, 512], F32, tag="mm")
                for kt in range(KT):
                    st, sp = (kt == 0), (kt == KT - 1)
                    mv = nxT[:, kt * TOK + to:kt * TOK + to + 512]
                    nc.tensor.matmul(psA[:], wbt[0][:, kt * 128:(kt + 1) * 128], mv,
                                     start=st, stop=sp)
                    nc.tensor.matmul(psB[:], wbt[1][:, kt * 128:(kt + 1) * 128], mv,
                                     start=st, stop=sp)
                for (ci, r, bi), ps in ((blkA, psA), (blkB, psB)):
                    ghb = r * HBL + OFFS[ci] + bi
                    hs = psc.tile([128, 512], F32, tag="hs")
                    nc.vector.tensor_tensor(hs[:], ps[:], s1r[:, to:to + 512], op=OP.mult)
                    hsl = htile[:, ghb * 512:(ghb + 1) * 512]
                    nc.scalar.activation(hsl, hs[:], AF.Gelu, bias=zero_col[:])
                    ga = psc.tile([128, 512], BF16, tag="ga")
                    nc.scalar.activation(ga[:], hsl, AF.Abs, bias=zero_col[:])
                    nc.vector.tensor_tensor(acc[:, to:to + 512], acc[:, to:to + 512],
                                            ga[:], op=OP.max)

            for (blkA, blkB) in pairs:
                mm1_pair(blkA, blkB, 0, h0)

        # ---- phase A scratch pool closed; h1 + late rows live in its space --
        with ExitStack() as sb:
            hp1 = sb.enter_context(tc.tile_pool(name="hp1", bufs=1))
            rowp = sb.enter_context(tc.tile_pool(name="rowp", bufs=1))
            h1 = hp1.tile([128, HB * 512], BF16)
            # vector-only scratch rows, both on partition 0 (engine AP rule);
            # the s2/i2 rows themselves are computed into partition 0 of their
            # broadcast tiles.
            lrows = rowp.tile([1, 1024], F32)

            def half_rows(th):
                """acc[:, th*512:+512] absmax -> s2row/i2row for that half."""
                to = th * 512
                for c in range(4):
                    pt = ps_tr.tile([128, 128], F32, tag="tr")
                    nc.tensor.transpose(pt[:], acc[:, to + c * 128:to + (c + 1) * 128],
                                        ident[:])
                    nc.vector.tensor_reduce(m2t[:, c:c + 1], pt[:], axis=AX.X, op=OP.max)
                nc.sync.dma_start(r2_d[th][:].rearrange("c p -> p c"), m2t[:])
                m2row = lrows[0:1, 0:512]
                nc.sync.dma_start(m2row, r2_d[th][:].rearrange("c p -> (c p)")[None, :])
                nc.vector.tensor_scalar(m2row, m2row, 1e-5, None, op0=OP.max)
                rs2 = lrows[0:1, 512:1024]
                s2row = s2r[0:1, to:to + 512]
                nc.vector.reciprocal_approx_accurate(s2row, m2row, rs2)
                nc.vector.tensor_scalar(s2row, s2row, 127.0, None, op0=OP.mult)
                i2row = i2r[0:1, to:to + 512]
                nc.vector.reciprocal_approx_accurate(i2row, s2row, rs2)
                nc.vector.tensor_scalar(i2row, i2row, invw[:, 1:2], None, op0=OP.mult)
                return s2row, i2row

            def half_bcast(th, s2row, i2row):
                to = th * 512
                bcast_row(s2r, s2row, 512, off=to)
                bcast_row(i2r, i2row, 512, off=to)

            def q2_block(th, htile, kg):
                """requant h in place: vector scale-mult, scalar MAGIC round."""
                to = th * 512
                hsl = htile[:, kg * 512:(kg + 1) * 512]
                t2s = psc.tile([128, 512], F32, tag="hs")
                nc.vector.tensor_tensor(t2s[:], hsl, s2r[:, to:to + 512], op=OP.mult)
                rq = psc.tile([128, 512], F32, tag="hs")
                nc.scalar.activation(rq[:], t2s[:], AF.Copy, bias=MAGIC)
                nc.scalar.activation(hsl, rq[:], AF.Copy, bias=-MAGIC)

            # ============ MM1 half B, with rows(A) + Q2(A) hidden under it ===
            q2a = 0
            rowsA = [None]
            for idx, (blkA, blkB) in enumerate(pairs):
                mm1_pair(blkA, blkB, 1, h1)
                if idx == 2:
                    rowsA[0] = half_rows(0)
                elif idx == 4:
                    half_bcast(0, *rowsA[0])
                elif idx >= 5 and q2a < HB:
                    for _ in range(3):
                        if q2a < HB:
                            q2_block(0, h0, q2a)
                            q2a += 1
            while q2a < HB:
                q2_block(0, h0, q2a)
                q2a += 1

            # ============ MM2 per half, d-pairs with alternating PSUM banks ==
            def mm2_dpair(th, htile, d0):
                to = th * 512
                psA = ps_mm.tile([128, 512], F32, tag="mm")
                psB = ps_mm.tile([128, 512], F32, tag="mm")
                for r in range(n_cores):
                    wv0 = pmm2w.tile([128, HBL * 128], FP8, tag="wv")
                    nc.scalar.dma_start(wv0[:], t2_g[r, d0])
                    wv1 = pmm2w.tile([128, HBL * 128], FP8, tag="wv")
                    nc.sync.dma_start(wv1[:], t2_g[r, d0 + 1])
                    for k2 in range(HBL):
                        kg = r * HBL + k2
                        st, sp = (kg == 0), (kg == HB - 1)
                        mv = htile[:, kg * 512:(kg + 1) * 512]
                        nc.tensor.matmul(psA[:], wv0[:, k2 * 128:(k2 + 1) * 128], mv,
                                         start=st, stop=sp)
                        nc.tensor.matmul(psB[:], wv1[:, k2 * 128:(k2 + 1) * 128], mv,
                                         start=st, stop=sp)
                for d, ps in ((d0, psA), (d0 + 1, psB)):
                    ot = psc.tile([128, 512], F32, tag="hs")
                    nc.vector.tensor_tensor(ot[:], ps[:], i2r[:, to:to + 512], op=OP.mult)
                    nc.gpsimd.dma_start(outT[d * 128:(d + 1) * 128, to:to + 512], ot[:])

            q2b = 0
            rowsB = [None]
            for dp in range(DB // 2):
                mm2_dpair(0, h0, 2 * dp)
                if dp == 0:
                    rowsB[0] = half_rows(1)
                elif dp == 1:
                    half_bcast(1, *rowsB[0])
                elif q2b < HB:
                    for _ in range(11):
                        if q2b < HB:
                            q2_block(1, h1, q2b)
                            q2b += 1
            while q2b < HB:
                q2_block(1, h1, q2b)
                q2b += 1

            for dp in range(DB // 2):
                mm2_dpair(1, h1, 2 * dp)

    nc.compile()
    return nc


def _get_nc():
    if "nc" not in _cache:
        _cache["nc"] = _build()
    return _cache["nc"]


def _prep_inputs(x, w1, w2, gamma):
    x2d = np.ascontiguousarray(np.asarray(x, dtype=np.float32).reshape(NTOK, DIM))
    w1 = np.asarray(w1, dtype=np.float32)
    w2 = np.asarray(w2, dtype=np.float32)
    gamma = np.asarray(gamma, dtype=np.float32)
    w1T = np.ascontiguousarray(w1.T)          # [DIM, HID]
    w2T = np.ascontiguousarray(w2.T)          # [HID, DIM]
    gpt = np.ascontiguousarray(gamma.reshape(KT, 128).T)
    hs = HID // NCORES
    in_maps = []
    for c in range(NCORES):
        in_maps.append({
            "xT": np.ascontiguousarray(x2d[c * TOK:(c + 1) * TOK, :].T),
            "w1s": np.ascontiguousarray(w1T[:, c * hs:(c + 1) * hs]),
            "w2s": np.ascontiguousarray(w2T[c * hs:(c + 1) * hs, :]),
            "gpt": gpt,
        })
    return in_maps


def _run(in_maps, trace=False, **kw):
    nc = _get_nc()
    return bass_utils.run_bass_kernel_spmd(
        nc, in_maps, core_ids=list(range(NCORES)), trace=trace, **kw)


def kernel(x, w1, w2, gamma):
    in_maps = _prep_inputs(x, w1, w2, gamma)
    res = _run(in_maps, trace=False)
    out = np.empty((NTOK, DIM), dtype=np.float32)
    for c in range(NCORES):
        out[c * TOK:(c + 1) * TOK, :] = res.results[c]["outT"].T
    return out.reshape(B, S, DIM)
